# revision 19
# baseline (speedup 1.0000x reference)
"""Causal multi-head attention (B=1, S=4096, D=2048, H=16) on 8 trn2 cores.

Sharding: tensor-parallel over heads (2 heads/core) for QKV + attention;
output projection is head-sharded (row-parallel wo) with a per-head
AllToAll over sequence rows; the host concatenates the 8 row-slices.

v3 design notes (PE sustained clock is externally throttled to 13/16 =
1.95GHz; exec time ~= matmul-moving-columns/1.95GHz + stalls, so the
design minimizes columns and keeps every engine queue short):
 - causal 128-granularity on the diagonal 512-blocks: score/PV matmuls and
   exps are width-restricted; diagonal prob tiles have statically-zeroed
   garbage regions (memset once) so no mask tensors are needed at all.
 - softmax denominator: pairwise DVE tree over [128,1024] bf16 prob tiles,
   then gpsimd partition_all_reduce (3.5us, idle engine) + DVE reciprocal.
   No ones-matmul, no broadcast matmul -- zero PE cycles.
 - RoPE entirely on DVE via partition-offset muls (sin table is [-sin;sin]
   stacked), no ACT rot-copies: ACT runs only psum drains + exps, since
   phase-B exp throughput (~1.0us per [128,1024]) is at parity with PE.
 - PSUM is bank-granular: A uses psQK(4)+psV(2); sb7 runs v-pass then
   qk-pass so pools close early and phase B's ss2 (bufs=3) starts with no
   write-after-read stall; B overlaps A's tail via qb5/qb6 (ready after
   sb6).  Interleaved j-chunks share a V bank via start-once accumulation.
 - phase C is ss-serial (8-matmul groups, 4-bank double buffering), head-
   split so head-0's 33us of matmuls cover head-1's AllToAll; wo rows are
   prefetched on the ACT dma queue during phase B.
"""

import sys

for _p in ("/opt/trn_rl_repo", "/root/.axon_site/_ro/trn_rl_repo"):
    if _p not in sys.path:
        sys.path.insert(0, _p)

import numpy as np

import concourse.bacc as bacc
import concourse.mybir as mybir
from concourse import bass_isa
from concourse.bass_utils import run_bass_kernel_spmd
from concourse.tile import TileContext

F32 = mybir.dt.float32
BF16 = mybir.dt.bfloat16

S = 4096
D = 2048
H = 16
HD = 128
NCORES = 8
HPC = H // NCORES  # heads per core = 2
CPC = HPC * HD  # cols per core = 256
ROPE_THETA = 10000.0
SCALE = 1.0 / float(np.sqrt(np.float32(HD)))

NSB = S // 512  # 8 seq blocks of 512
NDT = D // 128  # 16 contraction tiles


def _rope_pair(nc, dst, p, cosS, sinS, pool):
    """dst = p*cos + rot(p)*sin, rot swaps partition halves; sinS rows are
    [sin; -sin] (pre-swapped) so each offset mul reads both SBUF inputs at
    the SAME base partition (BIR constraint) while the output is offset.
    4 DVE ops, no ACT."""
    t1 = pool.tile([128, 512], F32, tag="t1", name="t1")
    nc.vector.tensor_mul(t1[:], p[:], cosS[:])
    t2 = pool.tile([128, 512], F32, tag="t2", name="t2")
    nc.vector.tensor_mul(t2[0:64, :], p[64:128, :], sinS[64:128, :])
    nc.vector.tensor_mul(t2[64:128, :], p[0:64, :], sinS[0:64, :])
    nc.vector.tensor_add(dst[:], t1[:], t2[:])


def _emit_sb(nc, tensors, pools, sb, part):
    """Phase A for one seq block.  part: 'both' | 'v' | 'qk' (sb7 split).
    Weights are already streaming into w_r; xr tiles come from the xin pool.
    qk psum -> psb tiles (drained on DVE for sb7, ACT otherwise) + RoPE on
    DVE; v psum -> v_nat via ACT."""
    xT = tensors["xT"]
    cosf, sinf = tensors["cosf"], tensors["sinf"]
    w_r = tensors["w_r"]
    qT, kT, v_nat = tensors["qT"], tensors["kT"], tensors["v_nat"]
    xin, trig, rope, psQK, psV = (
        pools["xin"],
        pools["trig"],
        pools["rope"],
        pools.get("psQK"),
        pools.get("psV"),
    )
    sl = slice(sb * 512, (sb + 1) * 512)

    do_qk = part in ("both", "qk")
    do_v = part in ("both", "v")

    if do_qk:
        cosS = trig.tile([128, 512], F32, tag="cosS", name="cosS")
        sinS = trig.tile([128, 512], F32, tag="sinS", name="sinS")
        nc.sync.dma_start(out=cosS[:], in_=cosf[:, sl])
        nc.sync.dma_start(out=sinS[:], in_=sinf[:, sl])
        ps = {
            t: psQK.tile([128, 512], F32, tag=f"ps_{t}", name=f"ps_{t}")
            for t in ("q0", "q1", "k0", "k1")
        }
    if do_v:
        vps = [
            psV.tile([128, 512], F32, tag=f"ps_v{j2}", name=f"ps_v{j2}")
            for j2 in range(2)
        ]

    for d in range(NDT):
        xr = xin.tile([128, 512], BF16, tag="xr", name="xr")
        nc.sync.dma_start(out=xr[:], in_=xT[d * 128 : (d + 1) * 128, sl])
        st = dict(start=(d == 0), stop=(d == NDT - 1))
        if do_qk:
            for h in range(HPC):
                co = d * CPC + h * HD
                nc.tensor.matmul(
                    ps[f"q{h}"][:], w_r["q"][:, co : co + HD], xr[:], **st
                )
                nc.tensor.matmul(
                    ps[f"k{h}"][:], w_r["k"][:, co : co + HD], xr[:], **st
                )
        if do_v:
            for j in range(4):
                # two j-chunks share a psum bank: only the bank's first
                # matmul clears has_written (start-once), the second chunk's
                # d==0 matmul lands on clear bits and overwrites per-element
                nc.tensor.matmul(
                    vps[j // 2][:, (j % 2) * 256 : (j % 2) * 256 + 256],
                    xr[:, j * 128 : (j + 1) * 128],
                    w_r["v"][:, d * CPC : (d + 1) * CPC],
                    start=(d == 0 and j % 2 == 0),
                    stop=(d == NDT - 1),
                    skip_group_check=True,
                )

    if do_v:
        for j2 in range(2):
            nc.scalar.copy(v_nat[sb][:, j2 * 512 : (j2 + 1) * 512], vps[j2][:])
    if do_qk:
        psb = {}
        for h in range(HPC):
            for t in (f"q{h}", f"k{h}"):
                pt = rope.tile([128, 512], F32, tag=f"psb_{t}", name=f"psb_{t}", bufs=1)
                if part == "qk":  # sb7: ACT is needed for phase-B exps
                    nc.vector.tensor_copy(pt[:], ps[t][:])
                else:
                    nc.scalar.copy(pt[:], ps[t][:])
                psb[t] = pt
        return psb, cosS, sinS
    return None, None, None


def _emit_rope_sb(nc, tensors, pools, sb, psb, cosS, sinS, heads):
    qT, kT = tensors["qT"], tensors["kT"]
    rope = pools["rope"]
    for h in heads:
        _rope_pair(nc, qT[h][sb], psb[f"q{h}"], cosS, sinS, rope)
        _rope_pair(nc, kT[h][sb], psb[f"k{h}"], cosS, sinS, rope)


class _PhaseB:
    """Per-(head, q-block) causal attention with 2-group exp lookahead,
    width-restricted diagonal, pairwise denominator tree, gpsimd all-reduce
    normalization, and deferred flush for a2a overlap."""

    def __init__(self, nc, tensors, pools, a2a_ins, a2a_outs, groups, a2a_r):
        self.nc = nc
        self.t = tensors
        self.p = pools
        self.a2a_ins = a2a_ins
        self.a2a_outs = a2a_outs
        self.groups = groups
        self.a2a_r = a2a_r
        self.pending = None

    def flush(self):
        nc = self.nc
        if self.pending is None:
            return
        fh, fqb, fpo, fprS = self.pending
        self.pending = None
        nrm, attnP, a2asb = self.p["nrm"], self.p["attn"], self.p["a2asb"]
        denf = nrm.tile([128, 512], F32, tag="denf", name="denf")
        nc.gpsimd.partition_all_reduce(
            denf[:], fprS[:], channels=128, reduce_op=bass_isa.ReduceOp.add
        )
        rcsf = nrm.tile([128, 512], F32, tag="rcsf", name="rcsf")
        nc.vector.reciprocal_approx_fast(out=rcsf[:], in_=denf[:])
        at = attnP.tile([128, 512], BF16, tag="at", name="at")
        nc.vector.tensor_mul(at[:], fpo[:], rcsf[:])
        nc.sync.dma_start(
            out=self.a2a_ins[fh][fqb * 128 : (fqb + 1) * 128, :], in_=at[:]
        )
        if fqb == 0:  # qb0 is processed last per head
            nc.gpsimd.collective_compute(
                "AllToAll",
                mybir.AluOpType.bypass,
                replica_groups=self.groups,
                ins=[self.a2a_ins[fh].opt()],
                outs=[self.a2a_outs[fh].opt()],
            )
            ar = a2asb.tile(
                [128, NCORES * 512], BF16, tag=f"a2r{fh}", name=f"a2r{fh}"
            )
            for c in range(NCORES):
                nc.sync.dma_start(
                    out=ar[:, c * 512 : (c + 1) * 512],
                    in_=self.a2a_outs[fh][c * 128 : (c + 1) * 128, :],
                )
            self.a2a_r.append(ar)

    def emit_qb(self, h, qb):
        nc = self.nc
        t, p = self.t, self.p
        kT, qT, v_nat = t["kT"], t["qT"], t["v_nat"]
        psS, probs, treeP, fold = (
            p["psS"],
            p["probs"],
            p["tree"],
            p["fold"],
        )
        G = 2 * qb + 2
        po = psS.tile([128, 512], F32, tag="po", name="po", bufs=2)

        # per-group kt offsets (causal restriction on the diagonal block)
        def kt_off(g, half):
            kt = 2 * g + half
            if kt >= 4 * qb:  # diagonal 512-block
                return (kt - 4 * qb) * 128
            return 0

        ss2s = {}
        pr2s = {}

        def emit_sc(g):
            ss2 = psS.tile([128, 1024], F32, tag="ss2", name="ss2")
            ss2s[g] = ss2
            for half in range(2):
                kt = 2 * g + half
                off = kt_off(g, half)
                nc.tensor.matmul(
                    ss2[:, half * 512 + off : (half + 1) * 512],
                    kT[h][kt // 4][:, (kt % 4) * 128 : (kt % 4 + 1) * 128],
                    qT[h][qb][:, off:512],
                    start=True,
                    stop=True,
                )

        def emit_exp(g):
            ss2 = ss2s.pop(g)
            diag = kt_off(g, 1) > 0
            if not diag:
                pr2 = probs.tile([128, 1024], BF16, tag="pr2", name="pr2", bufs=4)
                nc.scalar.activation(
                    pr2[:], ss2[:], mybir.ActivationFunctionType.Exp, scale=SCALE
                )
            else:
                dt_tag = "pr2d0" if kt_off(g, 0) == 0 else "pr2d1"
                pr2 = probs.tile(
                    [128, 1024], BF16, tag=dt_tag, name=dt_tag, bufs=2
                )
                tri = self.t["trimask"]
                for half in range(2):
                    off = kt_off(g, half)
                    nc.scalar.activation(
                        pr2[:, half * 512 + off : (half + 1) * 512],
                        ss2[:, half * 512 + off : (half + 1) * 512],
                        mybir.ActivationFunctionType.Exp,
                        scale=SCALE,
                    )
                    # causal triangle within the first 128 q-cols of this kt
                    reg = pr2[:, half * 512 + off : half * 512 + off + 128]
                    nc.gpsimd.tensor_mul(reg, reg, tri[:])
            pr2s[g] = pr2

        def emit_pv(g):
            pr2 = pr2s[g]
            for half in range(2):
                kt = 2 * g + half
                off = kt_off(g, half)
                nc.tensor.matmul(
                    po[:, off:512],
                    v_nat[kt // 4][
                        :, ((kt % 4) * 2 + h) * 128 : ((kt % 4) * 2 + h + 1) * 128
                    ],
                    pr2[:, half * 512 + off : (half + 1) * 512],
                    start=(g == 0 and half == 0),
                    stop=(g == G - 1 and half == 1),
                    skip_group_check=True,
                )

        tree = []  # (level, tile[128,1024])

        def tree_push(g):
            if g % 2 == 0:
                return
            s = treeP.tile([128, 1024], BF16, tag="tl0", name="tl0", bufs=2)
            nc.vector.tensor_add(s[:], pr2s.pop(g - 1)[:], pr2s.pop(g)[:])
            tree.append((1, s))
            while len(tree) >= 2 and tree[-1][0] == tree[-2][0]:
                l1, t1 = tree.pop()
                _, t2 = tree.pop()
                tn = treeP.tile(
                    [128, 1024], BF16, tag=f"tl{l1}", name=f"tl{l1}", bufs=2
                )
                nc.vector.tensor_add(tn[:], t1[:], t2[:])
                tree.append((l1 + 1, tn))

        emit_sc(0)
        emit_exp(0)
        if G > 1:
            emit_sc(1)
            emit_exp(1)
        for g in range(G):
            if g == 1:
                self.flush()
            if g + 2 < G:
                emit_sc(g + 2)
            emit_pv(g)
            tree_push(g)
            if g + 2 < G:
                emit_exp(g + 2)
        while len(tree) > 1:
            l1, t1 = tree.pop()
            _, t2 = tree.pop()
            tn = treeP.tile([128, 1024], BF16, tag="tlc", name="tlc", bufs=2)
            nc.vector.tensor_add(tn[:], t1[:], t2[:])
            tree.append((l1 + 1, tn))
        root = tree[0][1]
        prS = fold.tile([128, 512], BF16, tag="prS", name="prS")
        nc.vector.tensor_add(prS[:], root[:, 0:512], root[:, 512:1024])
        self.pending = (h, qb, po, prS)


def _emit_wo_dma(nc, tensors, pools, h, dbp, c, queue):
    """One wo row-block DMA into the wosb pool (16 slots; h0's 16 tiles are
    prefetched on the gpsimd queue during phase B, h1's on ACT during C)."""
    wosb = pools["wosb"]
    wo_full = tensors["wo_full"]
    wr = wosb.tile([128, 1024], BF16, tag="wr", name="wr", bufs=13)
    ro = c * CPC + h * HD
    queue.dma_start(
        out=wr[:], in_=wo_full[ro : ro + 128, dbp * 1024 : (dbp + 1) * 1024]
    )
    return wr


def _phase_c(nc, tensors, pools, a2a_r, out_part, wrs):
    """Out-projection, ss-serial: per (h, dbp, j, ss) an 8-matmul psum group
    over source cores; h0 drains to bf16 SBUF acc, h1 adds + streams out."""
    psC, oacc, osb = (
        pools["psC"],
        pools["oacc"],
        pools["osb"],
    )
    # remaining wo rows stream in on the ACT dma queue while h0 computes
    for h in range(HPC):
        for dbp in range(2):
            for c in range(NCORES):
                if (h, dbp, c) not in wrs:
                    wrs[(h, dbp, c)] = _emit_wo_dma(
                        nc, tensors, pools, h, dbp, c, nc.scalar
                    )

    acc = {}
    for h in range(HPC):
        for dbp in range(2):
            for j in range(2):
                db = dbp * 2 + j
                for ss in range(4):
                    pc = psC.tile([128, 512], F32, tag="pc", name="pc", bufs=4)
                    for c in range(NCORES):
                        nc.tensor.matmul(
                            pc[:],
                            a2a_r[h][
                                :, c * 512 + ss * 128 : c * 512 + (ss + 1) * 128
                            ],
                            wrs[(h, dbp, c)][:, j * 512 : (j + 1) * 512],
                            start=(c == 0),
                            stop=(c == NCORES - 1),
                        )
                    if h == 0:
                        a = oacc.tile(
                            [128, 512], BF16, tag=f"acc{ss}{db}", name=f"acc{ss}{db}"
                        )
                        nc.scalar.copy(a[:], pc[:])
                        acc[(ss, db)] = a
                    else:
                        oc = osb.tile([128, 512], F32, tag="oc", name="oc")
                        nc.vector.tensor_add(oc[:], pc[:], acc[(ss, db)][:])
                        nc.sync.dma_start(
                            out=out_part[
                                ss * 128 : (ss + 1) * 128, db * 512 : (db + 1) * 512
                            ],
                            in_=oc[:],
                        )


def _build_program():
    nc = bacc.Bacc("TRN2", target_bir_lowering=False)

    tensors = {}
    tensors["xT"] = nc.dram_tensor("xT", [D, S], BF16, kind="ExternalInput")
    tensors["wq2"] = nc.dram_tensor("wq2", [D, CPC], BF16, kind="ExternalInput")
    tensors["wk2"] = nc.dram_tensor("wk2", [D, CPC], BF16, kind="ExternalInput")
    tensors["wv2"] = nc.dram_tensor("wv2", [D, CPC], BF16, kind="ExternalInput")
    tensors["wo_full"] = nc.dram_tensor("wo_full", [D, D], BF16, kind="ExternalInput")
    tensors["cosf"] = nc.dram_tensor("cosf", [HD, S], F32, kind="ExternalInput")
    tensors["sinf"] = nc.dram_tensor("sinf", [HD, S], F32, kind="ExternalInput")
    out_part = nc.dram_tensor("out_part", [S // NCORES, D], F32, kind="ExternalOutput")

    groups = [list(range(NCORES))]

    with TileContext(nc) as tc:
        with (
            tc.tile_pool(name="persist", bufs=1) as persist,
            tc.tile_pool(name="dram", bufs=1, space="DRAM") as dram,
            tc.tile_pool(name="xin", bufs=4) as xin,
            tc.tile_pool(name="trig", bufs=2) as trig,
            tc.tile_pool(name="rope", bufs=2) as rope,
            tc.tile_pool(name="probs", bufs=2) as probs,
            tc.tile_pool(name="tree", bufs=2) as treeP,
            tc.tile_pool(name="fold", bufs=2) as fold,
            tc.tile_pool(name="nrm", bufs=1) as nrm,
            tc.tile_pool(name="attn", bufs=2) as attnP,
            tc.tile_pool(name="a2asb", bufs=1) as a2asb,
            tc.tile_pool(name="wosb", bufs=16) as wosb,
            tc.tile_pool(name="oacc", bufs=1) as oacc,
            tc.tile_pool(name="osb", bufs=2) as osb,
        ):
            tensors["qT"] = [
                [
                    persist.tile([128, 512], BF16, tag=f"qT{h}_{sb}", name=f"qT{h}_{sb}")
                    for sb in range(NSB)
                ]
                for h in range(HPC)
            ]
            tensors["kT"] = [
                [
                    persist.tile([128, 512], BF16, tag=f"kT{h}_{sb}", name=f"kT{h}_{sb}")
                    for sb in range(NSB)
                ]
                for h in range(HPC)
            ]
            tensors["v_nat"] = [
                persist.tile([128, 1024], BF16, tag=f"v_nat{sb}", name=f"v_nat{sb}")
                for sb in range(NSB)
            ]
            # weights in SBUF, loaded in chunks on the ACT dma queue
            tensors["w_r"] = {
                wname: persist.tile(
                    [128, NDT * CPC], BF16, tag=f"w{wname}r", name=f"w{wname}r"
                )
                for wname in ("q", "k", "v")
            }

            # statically-zeroed garbage regions of the diagonal prob tiles
            # (allocated once per buf slot; exps never write these columns)
            for buf in range(2):
                pd0 = probs.tile([128, 1024], BF16, tag="pr2d0", name="pr2d0", bufs=2)
                nc.gpsimd.memset(pd0[:, 512:640], 0.0)
                pd1 = probs.tile([128, 1024], BF16, tag="pr2d1", name="pr2d1", bufs=2)
                nc.gpsimd.memset(pd1[:, 0:256], 0.0)
                nc.gpsimd.memset(pd1[:, 512:896], 0.0)

            # lower-triangle mask for the diagonal 128-blocks: keep col >= row
            trimask = persist.tile([128, 128], BF16, tag="trimask", name="trimask")
            nc.any.memset(trimask[:], 1.0)
            nc.gpsimd.affine_select(
                out=trimask[:],
                in_=trimask[:],
                compare_op=mybir.AluOpType.is_ge,
                fill=0.0,
                base=0,
                pattern=[[1, 128]],
                channel_multiplier=-1,
            )
            tensors["trimask"] = trimask

            a2a_ins = [
                dram.tile([NCORES * 128, 512], BF16, tag=f"a2i{h}", name=f"a2i{h}")
                for h in range(HPC)
            ]
            a2a_outs = [
                dram.tile([NCORES * 128, 512], BF16, tag=f"a2o{h}", name=f"a2o{h}")
                for h in range(HPC)
            ]

            pools = {
                "xin": xin,
                "trig": trig,
                "rope": rope,
                "probs": probs,
                "tree": treeP,
                "fold": fold,
                "nrm": nrm,
                "attn": attnP,
                "a2asb": a2asb,
            }

            def load_w_chunk(g, ndt):
                for wname in ("q", "k", "v"):
                    wv_ = tensors[f"w{wname}2"].rearrange("(dt p) c -> p dt c", p=128)
                    nc.scalar.dma_start(
                        out=tensors["w_r"][wname][
                            :, g * CPC : (g + ndt) * CPC
                        ].rearrange("p (dt c) -> p dt c", dt=ndt),
                        in_=wv_[:, g : g + ndt, :],
                    )

            pools["wosb"] = wosb
            pools["oacc"] = oacc
            pools["osb"] = osb
            a2a_r = []
            with tc.tile_pool(name="psQK", bufs=1, space="PSUM") as psQK:
                pools["psQK"] = psQK
                with tc.tile_pool(name="psV", bufs=1, space="PSUM") as psV:
                    pools["psV"] = psV
                    load_w_chunk(0, 1)
                    load_w_chunk(1, 3)
                    load_w_chunk(4, 4)
                    load_w_chunk(8, 4)
                    load_w_chunk(12, 4)
                    # sb0..sb6 fused qk+v
                    for sb in range(NSB - 1):
                        psb, cosS, sinS = _emit_sb(nc, tensors, pools, sb, "both")
                        _emit_rope_sb(
                            nc, tensors, pools, sb, psb, cosS, sinS, range(HPC)
                        )
                    # sb7 v-pass, then psV closes
                    _emit_sb(nc, tensors, pools, 7, "v")
                # sb7 qk-pass (psQK still open; drains go to DVE)
                psb7, cosS7, sinS7 = _emit_sb(nc, tensors, pools, 7, "qk")
            # psQK released -> psS gets its 4 banks + psV's 2 + 2 spares
            wrs = {}
            with tc.tile_pool(name="psS", bufs=3, space="PSUM") as psS:
                pools["psS"] = psS
                pb = _PhaseB(nc, tensors, pools, a2a_ins, a2a_outs, groups, a2a_r)

                def wo_pre(k2):
                    # 2 wo DMAs on the (mostly idle) gpsimd dma queue; only
                    # h0's 16 tiles prefetch here, so the 16-slot pool never
                    # forces a WAR wait on this queue mid-phase-B
                    for k in (2 * k2, 2 * k2 + 1):
                        if k < 13:
                            dbp, c = divmod(k, NCORES)
                            wrs[(0, dbp, c)] = _emit_wo_dma(
                                nc, tensors, pools, 0, dbp, c, nc.gpsimd
                            )

                # B-early: qb5 ready after sb6; overlaps sb7's drains
                pb.emit_qb(0, 5)
                _emit_rope_sb(nc, tensors, pools, 7, psb7, cosS7, sinS7, [0])
                wo_pre(0)
                pb.emit_qb(0, 6)
                wo_pre(1)
                pb.emit_qb(0, 7)
                wo_pre(2)
                pb.emit_qb(0, 1)
                _emit_rope_sb(nc, tensors, pools, 7, psb7, cosS7, sinS7, [1])
                wo_pre(3)
                for i, qb in enumerate((2, 3, 4, 0)):
                    pb.emit_qb(0, qb)
                    wo_pre(4 + i)
                for i, qb in enumerate((1, 2, 3, 4, 5, 6, 7, 0)):
                    pb.emit_qb(1, qb)
                    wo_pre(8 + i)
                pb.flush()
            with tc.tile_pool(name="psC", bufs=1, space="PSUM") as psC:
                pools["psC"] = psC
                _phase_c(nc, tensors, pools, a2a_r, out_part, wrs)

    nc.compile()
    return nc


_NC_CACHE = None


def _get_program():
    global _NC_CACHE
    if _NC_CACHE is None:
        _NC_CACHE = _build_program()
    return _NC_CACHE


def _rope_tables():
    # match reference's f32 arithmetic
    i = np.arange(0, HD, 2, dtype=np.float32) / np.float32(HD)
    freqs = (np.float32(1.0) / np.float32(ROPE_THETA) ** i).astype(np.float32)  # [64]
    ang = np.arange(S, dtype=np.float32)[:, None] * freqs[None, :]  # [S, 64]
    cos = np.cos(ang).astype(np.float32).T  # [64, S]
    sin = np.sin(ang).astype(np.float32).T
    cosf = np.concatenate([cos, cos], axis=0)  # [128, S]
    # pre-swapped for the partition-offset rope muls: rows [0:64] multiply
    # p[64:128] reading sinS[64:128] = -sin, rows [64:128] read [0:64] = +sin
    sinf = np.concatenate([sin, -sin], axis=0)
    return np.ascontiguousarray(cosf), np.ascontiguousarray(sinf)


def kernel(x, mask, wq, wk, wv, wo):
    # mask is the standard causal mask produced by setup_inputs; causality is
    # implemented directly in the device program, so the tensor itself is not
    # shipped to the cores.
    import ml_dtypes

    x = np.asarray(x, dtype=np.float32)
    wq = np.asarray(wq, dtype=np.float32)
    wk = np.asarray(wk, dtype=np.float32)
    wv = np.asarray(wv, dtype=np.float32)
    wo_b = np.ascontiguousarray(np.asarray(wo, dtype=np.float32).astype(ml_dtypes.bfloat16))

    xT = np.ascontiguousarray(
        x.reshape(S, D).T.astype(ml_dtypes.bfloat16)
    )  # [D, S] bf16

    # de-interleave permutation within each head (RoPE pairs -> halves)
    idx = np.concatenate([np.arange(0, HD, 2), np.arange(1, HD, 2)])
    perm = np.concatenate([h * HD + idx for h in range(H)])
    wq_p = wq[:, perm]
    wk_p = wk[:, perm]

    cosf, sinf = _rope_tables()

    nc = _get_program()
    in_maps = []
    for c in range(NCORES):
        csl = slice(c * CPC, (c + 1) * CPC)
        in_maps.append(
            {
                "xT": xT,
                "wq2": np.ascontiguousarray(wq_p[:, csl].astype(ml_dtypes.bfloat16)),
                "wk2": np.ascontiguousarray(wk_p[:, csl].astype(ml_dtypes.bfloat16)),
                "wv2": np.ascontiguousarray(wv[:, csl].astype(ml_dtypes.bfloat16)),
                "wo_full": wo_b,
                "cosf": cosf,
                "sinf": sinf,
            }
        )
    res = run_bass_kernel_spmd(nc, in_maps, core_ids=list(range(NCORES)))
    out = np.concatenate([res.results[c]["out_part"] for c in range(NCORES)], axis=0)
    return out.reshape(1, S, D).astype(np.float32)


# revision 24
# speedup vs baseline: 1.0299x; 1.0299x over previous
"""Causal multi-head attention (B=1, S=4096, D=2048, H=16) on 8 trn2 cores.

Sharding: tensor-parallel over heads (2 heads/core) for QKV + attention;
output projection is head-sharded (row-parallel wo) with a per-head
AllToAll over sequence rows; the host concatenates the 8 row-slices.

v3 design notes (PE sustained clock is externally throttled to 13/16 =
1.95GHz; exec time ~= matmul-moving-columns/1.95GHz + stalls, so the
design minimizes columns and keeps every engine queue short):
 - causal 128-granularity on the diagonal 512-blocks: score/PV matmuls and
   exps are width-restricted; diagonal prob tiles have statically-zeroed
   garbage regions (memset once) so no mask tensors are needed at all.
 - softmax denominator: pairwise DVE tree over [128,1024] bf16 prob tiles,
   then gpsimd partition_all_reduce (3.5us, idle engine) + DVE reciprocal.
   No ones-matmul, no broadcast matmul -- zero PE cycles.
 - RoPE entirely on DVE via partition-offset muls (sin table is [-sin;sin]
   stacked), no ACT rot-copies: ACT runs only psum drains + exps, since
   phase-B exp throughput (~1.0us per [128,1024]) is at parity with PE.
 - PSUM is bank-granular: A uses psQK(4)+psV(2); sb7 runs v-pass then
   qk-pass so pools close early and phase B's ss2 (bufs=3) starts with no
   write-after-read stall; B overlaps A's tail via qb5/qb6 (ready after
   sb6).  Interleaved j-chunks share a V bank via start-once accumulation.
 - phase C is ss-serial (8-matmul groups, 4-bank double buffering), head-
   split so head-0's 33us of matmuls cover head-1's AllToAll; wo rows are
   prefetched on the ACT dma queue during phase B.
"""

import sys

for _p in ("/opt/trn_rl_repo", "/root/.axon_site/_ro/trn_rl_repo"):
    if _p not in sys.path:
        sys.path.insert(0, _p)

import numpy as np

import concourse.bacc as bacc
import concourse.mybir as mybir
from concourse import bass_isa
from concourse.bass_utils import run_bass_kernel_spmd
from concourse.tile import TileContext

F32 = mybir.dt.float32
BF16 = mybir.dt.bfloat16

S = 4096
D = 2048
H = 16
HD = 128
NCORES = 8
HPC = H // NCORES  # heads per core = 2
CPC = HPC * HD  # cols per core = 256
ROPE_THETA = 10000.0
SCALE = 1.0 / float(np.sqrt(np.float32(HD)))

NSB = S // 512  # 8 seq blocks of 512
NDT = D // 128  # 16 contraction tiles


def _rope_pair(nc, dst, p, cosS, sinS, pool):
    """dst = p*cos + rot(p)*sin, rot swaps partition halves; sinS rows are
    [sin; -sin] (pre-swapped) so each offset mul reads both SBUF inputs at
    the SAME base partition (BIR constraint) while the output is offset.
    4 DVE ops, no ACT."""
    t1 = pool.tile([128, 512], F32, tag="t1", name="t1")
    nc.vector.tensor_mul(t1[:], p[:], cosS[:])
    t2 = pool.tile([128, 512], F32, tag="t2", name="t2")
    nc.vector.tensor_mul(t2[0:64, :], p[64:128, :], sinS[64:128, :])
    nc.vector.tensor_mul(t2[64:128, :], p[0:64, :], sinS[0:64, :])
    nc.vector.tensor_add(dst[:], t1[:], t2[:])


def _emit_sb(nc, tensors, pools, sb, part):
    """Phase A for one seq block.  part: 'both' | 'v' | 'qk' (sb7 split).
    Weights are already streaming into w_r; xr tiles come from the xin pool.
    qk psum -> psb tiles (drained on DVE for sb7, ACT otherwise) + RoPE on
    DVE; v psum -> v_nat via ACT."""
    xT = tensors["xT"]
    cosf, sinf = tensors["cosf"], tensors["sinf"]
    w_r = tensors["w_r"]
    qT, kT, v_nat = tensors["qT"], tensors["kT"], tensors["v_nat"]
    xin, trig, rope, psQK, psV = (
        pools["xin"],
        pools["trig"],
        pools["rope"],
        pools.get("psQK"),
        pools.get("psV"),
    )
    sl = slice(sb * 512, (sb + 1) * 512)

    do_qk = part in ("both", "qk")
    do_v = part in ("both", "v")

    if do_qk:
        cosS = trig.tile([128, 512], F32, tag="cosS", name="cosS")
        sinS = trig.tile([128, 512], F32, tag="sinS", name="sinS")
        if sb > 0:  # sb0's tables load after its xr stream (startup path)
            nc.sync.dma_start(out=cosS[:], in_=cosf[:, sl])
            nc.sync.dma_start(out=sinS[:], in_=sinf[:, sl])
        ps = {
            t: psQK.tile([128, 512], F32, tag=f"ps_{t}", name=f"ps_{t}")
            for t in ("q0", "q1", "k0", "k1")
        }
    if do_v:
        vps = [
            psV.tile([128, 512], F32, tag=f"ps_v{j2}", name=f"ps_v{j2}")
            for j2 in range(2)
        ]

    for d in range(NDT):
        xr = xin.tile([128, 512], BF16, tag="xr", name="xr")
        nc.sync.dma_start(out=xr[:], in_=xT[d * 128 : (d + 1) * 128, sl])
        st = dict(start=(d == 0), stop=(d == NDT - 1))
        if do_qk:
            for h in range(HPC):
                co = d * CPC + h * HD
                nc.tensor.matmul(
                    ps[f"q{h}"][:], w_r["q"][:, co : co + HD], xr[:], **st
                )
                nc.tensor.matmul(
                    ps[f"k{h}"][:], w_r["k"][:, co : co + HD], xr[:], **st
                )
        if do_v:
            for j in range(4):
                # two j-chunks share a psum bank: only the bank's first
                # matmul clears has_written (start-once), the second chunk's
                # d==0 matmul lands on clear bits and overwrites per-element
                nc.tensor.matmul(
                    vps[j // 2][:, (j % 2) * 256 : (j % 2) * 256 + 256],
                    xr[:, j * 128 : (j + 1) * 128],
                    w_r["v"][:, d * CPC : (d + 1) * CPC],
                    start=(d == 0 and j % 2 == 0),
                    stop=(d == NDT - 1),
                    skip_group_check=True,
                )

    if do_qk and sb == 0:
        nc.sync.dma_start(out=cosS[:], in_=cosf[:, sl])
        nc.sync.dma_start(out=sinS[:], in_=sinf[:, sl])
    if do_v:
        for j2 in range(2):
            nc.scalar.copy(v_nat[sb][:, j2 * 512 : (j2 + 1) * 512], vps[j2][:])
    if do_qk:
        psb = {}
        for h in range(HPC):
            for t in (f"q{h}", f"k{h}"):
                pt = rope.tile([128, 512], F32, tag=f"psb_{t}", name=f"psb_{t}", bufs=1)
                if part == "qk":  # sb7: ACT is needed for phase-B exps
                    nc.vector.tensor_copy(pt[:], ps[t][:])
                else:
                    nc.scalar.copy(pt[:], ps[t][:])
                psb[t] = pt
        return psb, cosS, sinS
    return None, None, None


def _emit_rope_sb(nc, tensors, pools, sb, psb, cosS, sinS, heads):
    qT, kT = tensors["qT"], tensors["kT"]
    rope = pools["rope"]
    for h in heads:
        _rope_pair(nc, qT[h][sb], psb[f"q{h}"], cosS, sinS, rope)
        _rope_pair(nc, kT[h][sb], psb[f"k{h}"], cosS, sinS, rope)


class _PhaseB:
    """Per-(head, q-block) causal attention with 2-group exp lookahead,
    width-restricted diagonal, pairwise denominator tree, gpsimd all-reduce
    normalization, and deferred flush for a2a overlap."""

    def __init__(self, nc, tensors, pools, a2a_ins, a2a_outs, groups, a2a_r):
        self.nc = nc
        self.t = tensors
        self.p = pools
        self.a2a_ins = a2a_ins
        self.a2a_outs = a2a_outs
        self.groups = groups
        self.a2a_r = a2a_r
        self.pending = None  # awaiting flush1 (all-reduce kickoff)
        self.pending2 = None  # awaiting flush2 (recip/at/dma)

    def flush1(self):
        """Kick the gpsimd all-reduce for the previous qb.  Only gpsimd-queue
        ops here -- nothing that could head-of-line block DVE."""
        nc = self.nc
        if self.pending is None:
            return
        fh, fqb, fpo, fprS = self.pending
        self.pending = None
        nrm = self.p["nrm"]
        denf = nrm.tile([128, 512], F32, tag="denf", name="denf")
        nc.gpsimd.partition_all_reduce(
            denf[:], fprS[:], channels=128, reduce_op=bass_isa.ReduceOp.add
        )
        self.pending2 = (fh, fqb, fpo, denf)

    def flush2(self):
        """DVE tail of the flush, emitted ~4 groups after flush1 so the
        reciprocal never waits on the all-reduce in the DVE FIFO."""
        nc = self.nc
        if self.pending2 is None:
            return
        fh, fqb, fpo, denf = self.pending2
        self.pending2 = None
        nrm, attnP, a2asb = self.p["nrm"], self.p["attn"], self.p["a2asb"]
        rcsf = nrm.tile([128, 512], F32, tag="rcsf", name="rcsf")
        nc.vector.reciprocal_approx_fast(out=rcsf[:], in_=denf[:])
        at = attnP.tile([128, 512], BF16, tag="at", name="at")
        nc.vector.tensor_mul(at[:], fpo[:], rcsf[:])
        nc.sync.dma_start(
            out=self.a2a_ins[fh][fqb * 128 : (fqb + 1) * 128, :], in_=at[:]
        )
        if fqb == 0:  # qb0 is processed last per head
            nc.gpsimd.collective_compute(
                "AllToAll",
                mybir.AluOpType.bypass,
                replica_groups=self.groups,
                ins=[self.a2a_ins[fh].opt()],
                outs=[self.a2a_outs[fh].opt()],
            )
            ar = a2asb.tile(
                [128, NCORES * 512], BF16, tag=f"a2r{fh}", name=f"a2r{fh}"
            )
            for c in range(NCORES):
                nc.sync.dma_start(
                    out=ar[:, c * 512 : (c + 1) * 512],
                    in_=self.a2a_outs[fh][c * 128 : (c + 1) * 128, :],
                )
            self.a2a_r.append(ar)

    def emit_qb(self, h, qb):
        nc = self.nc
        t, p = self.t, self.p
        kT, qT, v_nat = t["kT"], t["qT"], t["v_nat"]
        psS, probs, treeP, fold = (
            p["psS"],
            p["probs"],
            p["tree"],
            p["fold"],
        )
        G = 2 * qb + 2
        po = psS.tile([128, 512], F32, tag="po", name="po", bufs=2)

        # per-group kt offsets (causal restriction on the diagonal block)
        def kt_off(g, half):
            kt = 2 * g + half
            if kt >= 4 * qb:  # diagonal 512-block
                return (kt - 4 * qb) * 128
            return 0

        ss2s = {}
        pr2s = {}

        def emit_sc(g):
            ss2 = psS.tile([128, 1024], F32, tag="ss2", name="ss2")
            ss2s[g] = ss2
            for half in range(2):
                kt = 2 * g + half
                off = kt_off(g, half)
                nc.tensor.matmul(
                    ss2[:, half * 512 + off : (half + 1) * 512],
                    kT[h][kt // 4][:, (kt % 4) * 128 : (kt % 4 + 1) * 128],
                    qT[h][qb][:, off:512],
                    start=True,
                    stop=True,
                )

        def emit_exp(g):
            ss2 = ss2s.pop(g)
            diag = kt_off(g, 1) > 0
            if not diag:
                pr2 = probs.tile([128, 1024], BF16, tag="pr2", name="pr2", bufs=4)
                nc.scalar.activation(
                    pr2[:], ss2[:], mybir.ActivationFunctionType.Exp, scale=SCALE
                )
            else:
                dt_tag = "pr2d0" if kt_off(g, 0) == 0 else "pr2d1"
                pr2 = probs.tile(
                    [128, 1024], BF16, tag=dt_tag, name=dt_tag, bufs=2
                )
                tri = self.t["trimask"]
                for half in range(2):
                    off = kt_off(g, half)
                    nc.scalar.activation(
                        pr2[:, half * 512 + off : (half + 1) * 512],
                        ss2[:, half * 512 + off : (half + 1) * 512],
                        mybir.ActivationFunctionType.Exp,
                        scale=SCALE,
                    )
                    # causal triangle within the first 128 q-cols of this kt
                    reg = pr2[:, half * 512 + off : half * 512 + off + 128]
                    nc.gpsimd.tensor_mul(reg, reg, tri[:])
            pr2s[g] = pr2

        def emit_pv(g):
            pr2 = pr2s[g]
            for half in range(2):
                kt = 2 * g + half
                off = kt_off(g, half)
                nc.tensor.matmul(
                    po[:, off:512],
                    v_nat[kt // 4][
                        :, ((kt % 4) * 2 + h) * 128 : ((kt % 4) * 2 + h + 1) * 128
                    ],
                    pr2[:, half * 512 + off : (half + 1) * 512],
                    start=(g == 0 and half == 0),
                    stop=(g == G - 1 and half == 1),
                    skip_group_check=True,
                )

        tree = []  # (level, tile[128,1024])

        def tree_push(g):
            if g % 2 == 0:
                return
            s = treeP.tile([128, 1024], BF16, tag="tl0", name="tl0", bufs=2)
            nc.vector.tensor_add(s[:], pr2s.pop(g - 1)[:], pr2s.pop(g)[:])
            tree.append((1, s))
            while len(tree) >= 2 and tree[-1][0] == tree[-2][0]:
                l1, t1 = tree.pop()
                _, t2 = tree.pop()
                tn = treeP.tile(
                    [128, 1024], BF16, tag=f"tl{l1}", name=f"tl{l1}", bufs=2
                )
                nc.vector.tensor_add(tn[:], t1[:], t2[:])
                tree.append((l1 + 1, tn))

        emit_sc(0)
        emit_exp(0)
        if G > 1:
            emit_sc(1)
            emit_exp(1)
        for g in range(G):
            if g == 1:
                self.flush1()
            if g == min(5, G - 1):
                self.flush2()
            if g + 2 < G:
                emit_sc(g + 2)
            emit_pv(g)
            tree_push(g)
            if g + 2 < G:
                emit_exp(g + 2)
        while len(tree) > 1:
            l1, t1 = tree.pop()
            _, t2 = tree.pop()
            tn = treeP.tile([128, 1024], BF16, tag="tlc", name="tlc", bufs=2)
            nc.vector.tensor_add(tn[:], t1[:], t2[:])
            tree.append((l1 + 1, tn))
        root = tree[0][1]
        prS = fold.tile([128, 512], BF16, tag="prS", name="prS")
        nc.vector.tensor_add(prS[:], root[:, 0:512], root[:, 512:1024])
        self.pending = (h, qb, po, prS)


def _emit_wo_dma(nc, tensors, pools, h, dbp, c, queue):
    """One wo row-block DMA into the wosb pool (16 slots; h0's 16 tiles are
    prefetched on the gpsimd queue during phase B, h1's on ACT during C)."""
    wosb = pools["wosb"]
    wo_full = tensors["wo_full"]
    wr = wosb.tile([128, 1024], BF16, tag="wr", name="wr", bufs=13)
    ro = c * CPC + h * HD
    queue.dma_start(
        out=wr[:], in_=wo_full[ro : ro + 128, dbp * 1024 : (dbp + 1) * 1024]
    )
    return wr


def _phase_c(nc, tensors, pools, a2a_r, out_part, wrs):
    """Out-projection, ss-serial: per (h, dbp, j, ss) an 8-matmul psum group
    over source cores; h0 drains to bf16 SBUF acc, h1 adds + streams out."""
    psC, oacc, osb = (
        pools["psC"],
        pools["oacc"],
        pools["osb"],
    )
    # remaining wo rows stream in on the ACT dma queue while h0 computes
    for h in range(HPC):
        for dbp in range(2):
            for c in range(NCORES):
                if (h, dbp, c) not in wrs:
                    wrs[(h, dbp, c)] = _emit_wo_dma(
                        nc, tensors, pools, h, dbp, c, nc.scalar
                    )

    acc = {}
    for h in range(HPC):
        for dbp in range(2):
            for j in range(2):
                db = dbp * 2 + j
                for ss in range(4):
                    pc = psC.tile([128, 512], F32, tag="pc", name="pc", bufs=4)
                    for c in range(NCORES):
                        nc.tensor.matmul(
                            pc[:],
                            a2a_r[h][
                                :, c * 512 + ss * 128 : c * 512 + (ss + 1) * 128
                            ],
                            wrs[(h, dbp, c)][:, j * 512 : (j + 1) * 512],
                            start=(c == 0),
                            stop=(c == NCORES - 1),
                        )
                    if h == 0:
                        a = oacc.tile(
                            [128, 512], BF16, tag=f"acc{ss}{db}", name=f"acc{ss}{db}"
                        )
                        nc.scalar.copy(a[:], pc[:])
                        acc[(ss, db)] = a
                    else:
                        oc = osb.tile([128, 512], F32, tag="oc", name="oc")
                        nc.vector.tensor_add(oc[:], pc[:], acc[(ss, db)][:])
                        nc.sync.dma_start(
                            out=out_part[
                                ss * 128 : (ss + 1) * 128, db * 512 : (db + 1) * 512
                            ],
                            in_=oc[:],
                        )


def _build_program():
    nc = bacc.Bacc("TRN2", target_bir_lowering=False)

    tensors = {}
    tensors["xT"] = nc.dram_tensor("xT", [D, S], BF16, kind="ExternalInput")
    tensors["wq2"] = nc.dram_tensor("wq2", [D, CPC], BF16, kind="ExternalInput")
    tensors["wk2"] = nc.dram_tensor("wk2", [D, CPC], BF16, kind="ExternalInput")
    tensors["wv2"] = nc.dram_tensor("wv2", [D, CPC], BF16, kind="ExternalInput")
    tensors["wo_full"] = nc.dram_tensor("wo_full", [D, D], BF16, kind="ExternalInput")
    tensors["cosf"] = nc.dram_tensor("cosf", [HD, S], F32, kind="ExternalInput")
    tensors["sinf"] = nc.dram_tensor("sinf", [HD, S], F32, kind="ExternalInput")
    out_part = nc.dram_tensor("out_part", [S // NCORES, D], F32, kind="ExternalOutput")

    groups = [list(range(NCORES))]

    with TileContext(nc) as tc:
        with (
            tc.tile_pool(name="persist", bufs=1) as persist,
            tc.tile_pool(name="dram", bufs=1, space="DRAM") as dram,
            tc.tile_pool(name="xin", bufs=4) as xin,
            tc.tile_pool(name="trig", bufs=2) as trig,
            tc.tile_pool(name="rope", bufs=2) as rope,
            tc.tile_pool(name="probs", bufs=2) as probs,
            tc.tile_pool(name="tree", bufs=2) as treeP,
            tc.tile_pool(name="fold", bufs=2) as fold,
            tc.tile_pool(name="nrm", bufs=1) as nrm,
            tc.tile_pool(name="attn", bufs=2) as attnP,
            tc.tile_pool(name="a2asb", bufs=1) as a2asb,
            tc.tile_pool(name="wosb", bufs=16) as wosb,
            tc.tile_pool(name="oacc", bufs=1) as oacc,
            tc.tile_pool(name="osb", bufs=2) as osb,
        ):
            tensors["qT"] = [
                [
                    persist.tile([128, 512], BF16, tag=f"qT{h}_{sb}", name=f"qT{h}_{sb}")
                    for sb in range(NSB)
                ]
                for h in range(HPC)
            ]
            tensors["kT"] = [
                [
                    persist.tile([128, 512], BF16, tag=f"kT{h}_{sb}", name=f"kT{h}_{sb}")
                    for sb in range(NSB)
                ]
                for h in range(HPC)
            ]
            tensors["v_nat"] = [
                persist.tile([128, 1024], BF16, tag=f"v_nat{sb}", name=f"v_nat{sb}")
                for sb in range(NSB)
            ]
            # weights in SBUF, loaded in chunks on the ACT dma queue
            tensors["w_r"] = {
                wname: persist.tile(
                    [128, NDT * CPC], BF16, tag=f"w{wname}r", name=f"w{wname}r"
                )
                for wname in ("q", "k", "v")
            }

            # statically-zeroed garbage regions of the diagonal prob tiles
            # (allocated once per buf slot; exps never write these columns)
            for buf in range(2):
                pd0 = probs.tile([128, 1024], BF16, tag="pr2d0", name="pr2d0", bufs=2)
                nc.gpsimd.memset(pd0[:, 512:640], 0.0)
                pd1 = probs.tile([128, 1024], BF16, tag="pr2d1", name="pr2d1", bufs=2)
                nc.gpsimd.memset(pd1[:, 0:256], 0.0)
                nc.gpsimd.memset(pd1[:, 512:896], 0.0)

            # lower-triangle mask for the diagonal 128-blocks: keep col >= row
            trimask = persist.tile([128, 128], BF16, tag="trimask", name="trimask")
            nc.any.memset(trimask[:], 1.0)
            nc.gpsimd.affine_select(
                out=trimask[:],
                in_=trimask[:],
                compare_op=mybir.AluOpType.is_ge,
                fill=0.0,
                base=0,
                pattern=[[1, 128]],
                channel_multiplier=-1,
            )
            tensors["trimask"] = trimask

            a2a_ins = [
                dram.tile([NCORES * 128, 512], BF16, tag=f"a2i{h}", name=f"a2i{h}")
                for h in range(HPC)
            ]
            a2a_outs = [
                dram.tile([NCORES * 128, 512], BF16, tag=f"a2o{h}", name=f"a2o{h}")
                for h in range(HPC)
            ]

            pools = {
                "xin": xin,
                "trig": trig,
                "rope": rope,
                "probs": probs,
                "tree": treeP,
                "fold": fold,
                "nrm": nrm,
                "attn": attnP,
                "a2asb": a2asb,
            }

            def load_w_chunk(g, ndt):
                for wname in ("q", "k", "v"):
                    wv_ = tensors[f"w{wname}2"].rearrange("(dt p) c -> p dt c", p=128)
                    eng = nc.gpsimd if wname == "v" else nc.scalar
                    eng.dma_start(
                        out=tensors["w_r"][wname][
                            :, g * CPC : (g + ndt) * CPC
                        ].rearrange("p (dt c) -> p dt c", dt=ndt),
                        in_=wv_[:, g : g + ndt, :],
                    )

            pools["wosb"] = wosb
            pools["oacc"] = oacc
            pools["osb"] = osb
            a2a_r = []
            with tc.tile_pool(name="psQK", bufs=1, space="PSUM") as psQK:
                pools["psQK"] = psQK
                with tc.tile_pool(name="psV", bufs=1, space="PSUM") as psV:
                    pools["psV"] = psV
                    load_w_chunk(0, 1)
                    load_w_chunk(1, 3)
                    load_w_chunk(4, 4)
                    load_w_chunk(8, 4)
                    load_w_chunk(12, 4)
                    # sb0..sb6 fused qk+v
                    for sb in range(NSB - 1):
                        psb, cosS, sinS = _emit_sb(nc, tensors, pools, sb, "both")
                        _emit_rope_sb(
                            nc, tensors, pools, sb, psb, cosS, sinS, range(HPC)
                        )
                    # sb7 v-pass, then psV closes
                    _emit_sb(nc, tensors, pools, 7, "v")
                # sb7 qk-pass (psQK still open; drains go to DVE)
                psb7, cosS7, sinS7 = _emit_sb(nc, tensors, pools, 7, "qk")
            # psQK released -> psS gets its 4 banks + psV's 2 + 2 spares
            wrs = {}
            with tc.tile_pool(name="psS", bufs=3, space="PSUM") as psS:
                pools["psS"] = psS
                pb = _PhaseB(nc, tensors, pools, a2a_ins, a2a_outs, groups, a2a_r)

                def wo_pre(k2):
                    # 2 wo DMAs on the (mostly idle) gpsimd dma queue; only
                    # h0's 16 tiles prefetch here, so the 16-slot pool never
                    # forces a WAR wait on this queue mid-phase-B
                    for k in (2 * k2, 2 * k2 + 1):
                        if k < 13:
                            dbp, c = divmod(k, NCORES)
                            wrs[(0, dbp, c)] = _emit_wo_dma(
                                nc, tensors, pools, 0, dbp, c, nc.gpsimd
                            )

                # B-early: qb5 ready after sb6; overlaps sb7's drains
                pb.emit_qb(0, 5)
                _emit_rope_sb(nc, tensors, pools, 7, psb7, cosS7, sinS7, [0])
                wo_pre(0)
                pb.emit_qb(0, 6)
                wo_pre(1)
                pb.emit_qb(0, 7)
                wo_pre(2)
                pb.emit_qb(0, 1)
                _emit_rope_sb(nc, tensors, pools, 7, psb7, cosS7, sinS7, [1])
                wo_pre(3)
                for i, qb in enumerate((2, 3, 4, 0)):
                    pb.emit_qb(0, qb)
                    wo_pre(4 + i)
                for i, qb in enumerate((1, 2, 3, 4, 5, 6, 7, 0)):
                    pb.emit_qb(1, qb)
                    wo_pre(8 + i)
                pb.flush1()
                pb.flush2()
            with tc.tile_pool(name="psC", bufs=1, space="PSUM") as psC:
                pools["psC"] = psC
                _phase_c(nc, tensors, pools, a2a_r, out_part, wrs)

    nc.compile()
    return nc


_NC_CACHE = None


def _get_program():
    global _NC_CACHE
    if _NC_CACHE is None:
        _NC_CACHE = _build_program()
    return _NC_CACHE


def _rope_tables():
    # match reference's f32 arithmetic
    i = np.arange(0, HD, 2, dtype=np.float32) / np.float32(HD)
    freqs = (np.float32(1.0) / np.float32(ROPE_THETA) ** i).astype(np.float32)  # [64]
    ang = np.arange(S, dtype=np.float32)[:, None] * freqs[None, :]  # [S, 64]
    cos = np.cos(ang).astype(np.float32).T  # [64, S]
    sin = np.sin(ang).astype(np.float32).T
    cosf = np.concatenate([cos, cos], axis=0)  # [128, S]
    # pre-swapped for the partition-offset rope muls: rows [0:64] multiply
    # p[64:128] reading sinS[64:128] = -sin, rows [64:128] read [0:64] = +sin
    sinf = np.concatenate([sin, -sin], axis=0)
    return np.ascontiguousarray(cosf), np.ascontiguousarray(sinf)


def kernel(x, mask, wq, wk, wv, wo):
    # mask is the standard causal mask produced by setup_inputs; causality is
    # implemented directly in the device program, so the tensor itself is not
    # shipped to the cores.
    import ml_dtypes

    x = np.asarray(x, dtype=np.float32)
    wq = np.asarray(wq, dtype=np.float32)
    wk = np.asarray(wk, dtype=np.float32)
    wv = np.asarray(wv, dtype=np.float32)
    wo_b = np.ascontiguousarray(np.asarray(wo, dtype=np.float32).astype(ml_dtypes.bfloat16))

    xT = np.ascontiguousarray(
        x.reshape(S, D).T.astype(ml_dtypes.bfloat16)
    )  # [D, S] bf16

    # de-interleave permutation within each head (RoPE pairs -> halves)
    idx = np.concatenate([np.arange(0, HD, 2), np.arange(1, HD, 2)])
    perm = np.concatenate([h * HD + idx for h in range(H)])
    wq_p = wq[:, perm]
    wk_p = wk[:, perm]

    cosf, sinf = _rope_tables()

    nc = _get_program()
    in_maps = []
    for c in range(NCORES):
        csl = slice(c * CPC, (c + 1) * CPC)
        in_maps.append(
            {
                "xT": xT,
                "wq2": np.ascontiguousarray(wq_p[:, csl].astype(ml_dtypes.bfloat16)),
                "wk2": np.ascontiguousarray(wk_p[:, csl].astype(ml_dtypes.bfloat16)),
                "wv2": np.ascontiguousarray(wv[:, csl].astype(ml_dtypes.bfloat16)),
                "wo_full": wo_b,
                "cosf": cosf,
                "sinf": sinf,
            }
        )
    res = run_bass_kernel_spmd(nc, in_maps, core_ids=list(range(NCORES)))
    out = np.concatenate([res.results[c]["out_part"] for c in range(NCORES)], axis=0)
    return out.reshape(1, S, D).astype(np.float32)


# revision 31
# speedup vs baseline: 1.0926x; 1.0608x over previous
"""Causal multi-head attention (B=1, S=4096, D=2048, H=16) on 8 trn2 cores.

Sharding: tensor-parallel over heads (2 heads/core) for QKV + attention;
output projection is head-sharded (row-parallel wo) with a per-head
AllToAll over sequence rows; the host concatenates the 8 row-slices.

v3 design notes (PE sustained clock is externally throttled to 13/16 =
1.95GHz; exec time ~= matmul-moving-columns/1.95GHz + stalls, so the
design minimizes columns and keeps every engine queue short):
 - causal 128-granularity on the diagonal 512-blocks: score/PV matmuls and
   exps are width-restricted; diagonal prob tiles have statically-zeroed
   garbage regions (memset once) so no mask tensors are needed at all.
 - softmax denominator: pairwise DVE tree over [128,1024] bf16 prob tiles,
   then gpsimd partition_all_reduce (3.5us, idle engine) + DVE reciprocal.
   No ones-matmul, no broadcast matmul -- zero PE cycles.
 - RoPE entirely on DVE via partition-offset muls (sin table is [-sin;sin]
   stacked), no ACT rot-copies: ACT runs only psum drains + exps, since
   phase-B exp throughput (~1.0us per [128,1024]) is at parity with PE.
 - PSUM is bank-granular: A uses psQK(4)+psV(2); sb7 runs v-pass then
   qk-pass so pools close early and phase B's ss2 (bufs=3) starts with no
   write-after-read stall; B overlaps A's tail via qb5/qb6 (ready after
   sb6).  Interleaved j-chunks share a V bank via start-once accumulation.
 - phase C is ss-serial (8-matmul groups, 4-bank double buffering), head-
   split so head-0's 33us of matmuls cover head-1's AllToAll; wo rows are
   prefetched on the ACT dma queue during phase B.
"""

import sys

for _p in ("/opt/trn_rl_repo", "/root/.axon_site/_ro/trn_rl_repo"):
    if _p not in sys.path:
        sys.path.insert(0, _p)

import numpy as np

import concourse.bacc as bacc
import concourse.mybir as mybir
from concourse import bass_isa
from concourse.bass_utils import run_bass_kernel_spmd
from concourse.tile import TileContext

F32 = mybir.dt.float32
BF16 = mybir.dt.bfloat16

S = 4096
D = 2048
H = 16
HD = 128
NCORES = 8
HPC = H // NCORES  # heads per core = 2
CPC = HPC * HD  # cols per core = 256
ROPE_THETA = 10000.0
SCALE = 1.0 / float(np.sqrt(np.float32(HD)))

NSB = S // 512  # 8 seq blocks of 512
NDT = D // 128  # 16 contraction tiles


def _rope_pair(nc, dst, p, cosS, sinS, pool):
    """dst = p*cos + rot(p)*sin, rot swaps partition halves; sinS rows are
    [sin; -sin] (pre-swapped) so each offset mul reads both SBUF inputs at
    the SAME base partition (BIR constraint) while the output is offset.
    4 DVE ops, no ACT."""
    t1 = pool.tile([128, 512], F32, tag="t1", name="t1")
    nc.vector.tensor_mul(t1[:], p[:], cosS[:])
    t2 = pool.tile([128, 512], F32, tag="t2", name="t2")
    nc.vector.tensor_mul(t2[0:64, :], p[64:128, :], sinS[64:128, :])
    nc.vector.tensor_mul(t2[64:128, :], p[0:64, :], sinS[0:64, :])
    nc.vector.tensor_add(dst[:], t1[:], t2[:])


def _emit_sb(nc, tensors, pools, sb, part):
    """Phase A for one seq block.  part: 'both' | 'v' | 'qk' (sb7 split).
    Weights are already streaming into w_r; xr tiles come from the xin pool.
    qk psum -> psb tiles (drained on DVE for sb7, ACT otherwise) + RoPE on
    DVE; v psum -> v_nat via ACT."""
    xT = tensors["xT"]
    cosf, sinf = tensors["cosf"], tensors["sinf"]
    w_r = tensors["w_r"]
    qT, kT, v_nat = tensors["qT"], tensors["kT"], tensors["v_nat"]
    xin, trig, rope, psQK, psV = (
        pools["xin"],
        pools["trig"],
        pools["rope"],
        pools.get("psQK"),
        pools.get("psV"),
    )
    sl = slice(sb * 512, (sb + 1) * 512)

    do_qk = part in ("both", "qk")
    do_v = part in ("both", "v")

    if do_qk:
        cosS = trig.tile([128, 512], F32, tag="cosS", name="cosS")
        sinS = trig.tile([128, 512], F32, tag="sinS", name="sinS")
        if sb > 0:  # sb0's tables load after its xr stream (startup path)
            nc.sync.dma_start(out=cosS[:], in_=cosf[:, sl])
            nc.sync.dma_start(out=sinS[:], in_=sinf[:, sl])
        ps = {
            t: psQK.tile([128, 512], F32, tag=f"ps_{t}", name=f"ps_{t}")
            for t in ("q0", "q1", "k0", "k1")
        }
    if do_v:
        vps = [
            psV.tile([128, 512], F32, tag=f"ps_v{j2}", name=f"ps_v{j2}")
            for j2 in range(2)
        ]

    for d in range(NDT):
        xr = xin.tile([128, 512], BF16, tag="xr", name="xr")
        nc.sync.dma_start(out=xr[:], in_=xT[d * 128 : (d + 1) * 128, sl])
        wk_cb = tensors.get("wk_cb")
        if wk_cb is not None and part != "v":
            wk_cb(sb, d)
        st = dict(start=(d == 0), stop=(d == NDT - 1))
        if do_qk:
            for h in range(HPC):
                co = d * CPC + h * HD
                nc.tensor.matmul(
                    ps[f"q{h}"][:], w_r["q"][:, co : co + HD], xr[:], **st
                )
                nc.tensor.matmul(
                    ps[f"k{h}"][:], w_r["k"][:, co : co + HD], xr[:], **st
                )
        if do_v:
            for j in range(4):
                # two j-chunks share a psum bank: only the bank's first
                # matmul clears has_written (start-once), the second chunk's
                # d==0 matmul lands on clear bits and overwrites per-element
                nc.tensor.matmul(
                    vps[j // 2][:, (j % 2) * 256 : (j % 2) * 256 + 256],
                    xr[:, j * 128 : (j + 1) * 128],
                    w_r["v"][:, d * CPC : (d + 1) * CPC],
                    start=(d == 0 and j % 2 == 0),
                    stop=(d == NDT - 1),
                    skip_group_check=True,
                )

    if do_qk and sb == 0:
        nc.sync.dma_start(out=cosS[:], in_=cosf[:, sl])
        nc.sync.dma_start(out=sinS[:], in_=sinf[:, sl])
    if do_v:
        for j2 in range(2):
            nc.scalar.copy(v_nat[sb][:, j2 * 512 : (j2 + 1) * 512], vps[j2][:])
    if do_qk:
        psb = {}
        for h in range(HPC):
            for t in (f"q{h}", f"k{h}"):
                pt = rope.tile([128, 512], F32, tag=f"psb_{t}", name=f"psb_{t}", bufs=1)
                if part == "qk":  # sb7: ACT is needed for phase-B exps
                    nc.vector.tensor_copy(pt[:], ps[t][:])
                else:
                    nc.scalar.copy(pt[:], ps[t][:])
                psb[t] = pt
        return psb, cosS, sinS
    return None, None, None


def _emit_rope_sb(nc, tensors, pools, sb, psb, cosS, sinS, heads):
    qT, kT = tensors["qT"], tensors["kT"]
    rope = pools["rope"]
    for h in heads:
        _rope_pair(nc, qT[h][sb], psb[f"q{h}"], cosS, sinS, rope)
        _rope_pair(nc, kT[h][sb], psb[f"k{h}"], cosS, sinS, rope)


class _PhaseB:
    """Per-(head, q-block) causal attention with 2-group exp lookahead,
    width-restricted diagonal, pairwise denominator tree, gpsimd all-reduce
    normalization, and deferred flush for a2a overlap."""

    def __init__(self, nc, tensors, pools, a2a_ins, a2a_outs, groups, a2a_r):
        self.nc = nc
        self.t = tensors
        self.p = pools
        self.a2a_ins = a2a_ins
        self.a2a_outs = a2a_outs
        self.groups = groups
        self.a2a_r = a2a_r
        self.pending = None  # awaiting flush1 (all-reduce kickoff)
        self.pending2 = None  # awaiting flush2 (recip/at/dma)

    def flush1(self):
        """Kick the gpsimd all-reduce for the previous qb.  Only gpsimd-queue
        ops here -- nothing that could head-of-line block DVE."""
        nc = self.nc
        if self.pending is None:
            return
        fh, fqb, fpo, fprS = self.pending
        self.pending = None
        nrm = self.p["nrm"]
        denf = nrm.tile([128, 512], F32, tag="denf", name="denf")
        nc.gpsimd.partition_all_reduce(
            denf[:], fprS[:], channels=128, reduce_op=bass_isa.ReduceOp.add
        )
        self.pending2 = (fh, fqb, fpo, denf)

    def flush2(self):
        """DVE tail of the flush, emitted ~4 groups after flush1 so the
        reciprocal never waits on the all-reduce in the DVE FIFO."""
        nc = self.nc
        if self.pending2 is None:
            return
        fh, fqb, fpo, denf = self.pending2
        self.pending2 = None
        nrm, attnP, a2asb = self.p["nrm"], self.p["attn"], self.p["a2asb"]
        rcsf = nrm.tile([128, 512], F32, tag="rcsf", name="rcsf")
        nc.vector.reciprocal_approx_fast(out=rcsf[:], in_=denf[:])
        at = attnP.tile([128, 512], BF16, tag="at", name="at")
        nc.vector.tensor_mul(at[:], fpo[:], rcsf[:])
        nc.sync.dma_start(
            out=self.a2a_ins[fh][fqb * 128 : (fqb + 1) * 128, :], in_=at[:]
        )
        if fqb == 0:  # qb0 is processed last per head
            nc.gpsimd.collective_compute(
                "AllToAll",
                mybir.AluOpType.bypass,
                replica_groups=self.groups,
                ins=[self.a2a_ins[fh].opt()],
                outs=[self.a2a_outs[fh].opt()],
            )
            ar = a2asb.tile(
                [128, NCORES * 512], BF16, tag=f"a2r{fh}", name=f"a2r{fh}"
            )
            for c in range(NCORES):
                nc.sync.dma_start(
                    out=ar[:, c * 512 : (c + 1) * 512],
                    in_=self.a2a_outs[fh][c * 128 : (c + 1) * 128, :],
                )
            self.a2a_r.append(ar)

    def emit_qb(self, h, qb):
        nc = self.nc
        t, p = self.t, self.p
        kT, qT, v_nat = t["kT"], t["qT"], t["v_nat"]
        psS, probs, treeP, fold = (
            p["psS"],
            p["probs"],
            p["tree"],
            p["fold"],
        )
        G = 2 * qb + 2
        po = psS.tile([128, 512], F32, tag="po", name="po", bufs=2)

        # per-group kt offsets (causal restriction on the diagonal block)
        def kt_off(g, half):
            kt = 2 * g + half
            if kt >= 4 * qb:  # diagonal 512-block
                return (kt - 4 * qb) * 128
            return 0

        ss2s = {}
        pr2s = {}

        def emit_sc(g):
            ss2 = psS.tile([128, 1024], F32, tag="ss2", name="ss2")
            ss2s[g] = ss2
            for half in range(2):
                kt = 2 * g + half
                off = kt_off(g, half)
                diag = kt >= 4 * qb
                nc.tensor.matmul(
                    ss2[:, half * 512 + off : (half + 1) * 512],
                    kT[h][kt // 4][:, (kt % 4) * 128 : (kt % 4 + 1) * 128],
                    qT[h][qb][:, off:512],
                    start=True,
                    stop=not diag,
                    skip_group_check=diag,
                )
                if diag:
                    # accumulate -1e9 into the strictly-upper triangle of the
                    # first 128 q-cols (PE-side causal mask; exp -> exact 0)
                    nc.tensor.matmul(
                        ss2[:, half * 512 + off : half * 512 + off + 128],
                        self.t["ident"][:],
                        self.t["trineg"][:],
                        start=False,
                        stop=True,
                        skip_group_check=True,
                    )

        def emit_exp(g):
            ss2 = ss2s.pop(g)
            diag = kt_off(g, 1) > 0
            if not diag:
                pr2 = probs.tile([128, 1024], BF16, tag="pr2", name="pr2", bufs=4)
                nc.scalar.activation(
                    pr2[:], ss2[:], mybir.ActivationFunctionType.Exp, scale=SCALE
                )
            else:
                dt_tag = "pr2d0" if kt_off(g, 0) == 0 else "pr2d1"
                pr2 = probs.tile(
                    [128, 1024], BF16, tag=dt_tag, name=dt_tag, bufs=2
                )
                for half in range(2):
                    off = kt_off(g, half)
                    nc.scalar.activation(
                        pr2[:, half * 512 + off : (half + 1) * 512],
                        ss2[:, half * 512 + off : (half + 1) * 512],
                        mybir.ActivationFunctionType.Exp,
                        scale=SCALE,
                    )
            pr2s[g] = pr2

        def emit_pv(g):
            pr2 = pr2s[g]
            for half in range(2):
                kt = 2 * g + half
                off = kt_off(g, half)
                nc.tensor.matmul(
                    po[:, off:512],
                    v_nat[kt // 4][
                        :, ((kt % 4) * 2 + h) * 128 : ((kt % 4) * 2 + h + 1) * 128
                    ],
                    pr2[:, half * 512 + off : (half + 1) * 512],
                    start=(g == 0 and half == 0),
                    stop=(g == G - 1 and half == 1),
                    skip_group_check=True,
                )

        tree = []  # (level, tile[128,1024])

        def tree_push(g):
            if g % 2 == 0:
                return
            s = treeP.tile([128, 1024], BF16, tag="tl0", name="tl0", bufs=2)
            nc.vector.tensor_add(s[:], pr2s.pop(g - 1)[:], pr2s.pop(g)[:])
            tree.append((1, s))
            while len(tree) >= 2 and tree[-1][0] == tree[-2][0]:
                l1, t1 = tree.pop()
                _, t2 = tree.pop()
                tn = treeP.tile(
                    [128, 1024], BF16, tag=f"tl{l1}", name=f"tl{l1}", bufs=2
                )
                nc.vector.tensor_add(tn[:], t1[:], t2[:])
                tree.append((l1 + 1, tn))

        emit_sc(0)
        emit_exp(0)
        if G > 1:
            emit_sc(1)
            emit_exp(1)
        for g in range(G):
            if g == 1:
                self.flush1()
            if g == min(5, G - 1):
                self.flush2()
            if g + 2 < G:
                emit_sc(g + 2)
            emit_pv(g)
            tree_push(g)
            if g + 2 < G:
                emit_exp(g + 2)
        while len(tree) > 1:
            l1, t1 = tree.pop()
            _, t2 = tree.pop()
            tn = treeP.tile([128, 1024], BF16, tag="tlc", name="tlc", bufs=2)
            nc.vector.tensor_add(tn[:], t1[:], t2[:])
            tree.append((l1 + 1, tn))
        root = tree[0][1]
        prS = fold.tile([128, 512], BF16, tag="prS", name="prS")
        nc.vector.tensor_add(prS[:], root[:, 0:512], root[:, 512:1024])
        self.pending = (h, qb, po, prS)


def _emit_wo_dma(nc, tensors, pools, h, dbp, c, queue):
    """One wo row-block DMA into the wosb pool (16 slots; h0's 16 tiles are
    prefetched on the gpsimd queue during phase B, h1's on ACT during C)."""
    wosb = pools["wosb"]
    wo_full = tensors["wo_full"]
    wr = wosb.tile([128, 1024], BF16, tag="wr", name="wr", bufs=13)
    ro = c * CPC + h * HD
    queue.dma_start(
        out=wr[:], in_=wo_full[ro : ro + 128, dbp * 1024 : (dbp + 1) * 1024]
    )
    return wr


def _phase_c(nc, tensors, pools, a2a_r, out_part, wrs):
    """Out-projection, ss-serial: per (h, dbp, j, ss) an 8-matmul psum group
    over source cores; h0 drains to bf16 SBUF acc, h1 adds + streams out."""
    psC, oacc, osb = (
        pools["psC"],
        pools["oacc"],
        pools["osb"],
    )
    # remaining wo rows stream in on the ACT dma queue while h0 computes
    for h in range(HPC):
        for dbp in range(2):
            for c in range(NCORES):
                if (h, dbp, c) not in wrs:
                    wrs[(h, dbp, c)] = _emit_wo_dma(
                        nc, tensors, pools, h, dbp, c, nc.scalar
                    )

    acc = {}
    for h in range(HPC):
        for dbp in range(2):
            for j in range(2):
                db = dbp * 2 + j
                for ss in range(4):
                    pc = psC.tile([128, 512], F32, tag="pc", name="pc", bufs=4)
                    for c in range(NCORES):
                        nc.tensor.matmul(
                            pc[:],
                            a2a_r[h][
                                :, c * 512 + ss * 128 : c * 512 + (ss + 1) * 128
                            ],
                            wrs[(h, dbp, c)][:, j * 512 : (j + 1) * 512],
                            start=(c == 0),
                            stop=(c == NCORES - 1),
                        )
                    if h == 0:
                        a = oacc.tile(
                            [128, 512], BF16, tag=f"acc{ss}{db}", name=f"acc{ss}{db}"
                        )
                        nc.scalar.copy(a[:], pc[:])
                        acc[(ss, db)] = a
                    else:
                        oc = osb.tile([128, 512], F32, tag="oc", name="oc")
                        nc.vector.tensor_add(oc[:], pc[:], acc[(ss, db)][:])
                        nc.sync.dma_start(
                            out=out_part[
                                ss * 128 : (ss + 1) * 128, db * 512 : (db + 1) * 512
                            ],
                            in_=oc[:],
                        )


def _build_program():
    nc = bacc.Bacc("TRN2", target_bir_lowering=False)

    tensors = {}
    tensors["xT"] = nc.dram_tensor("xT", [D, S], BF16, kind="ExternalInput")
    tensors["wq2"] = nc.dram_tensor("wq2", [D, CPC], BF16, kind="ExternalInput")
    tensors["wk2"] = nc.dram_tensor("wk2", [D, CPC], BF16, kind="ExternalInput")
    tensors["wv2"] = nc.dram_tensor("wv2", [D, CPC], BF16, kind="ExternalInput")
    tensors["wo_full"] = nc.dram_tensor("wo_full", [D, D], BF16, kind="ExternalInput")
    tensors["cosf"] = nc.dram_tensor("cosf", [HD, S], F32, kind="ExternalInput")
    tensors["sinf"] = nc.dram_tensor("sinf", [HD, S], F32, kind="ExternalInput")
    out_part = nc.dram_tensor("out_part", [S // NCORES, D], F32, kind="ExternalOutput")

    groups = [list(range(NCORES))]

    with TileContext(nc) as tc:
        with (
            tc.tile_pool(name="persist", bufs=1) as persist,
            tc.tile_pool(name="dram", bufs=1, space="DRAM") as dram,
            tc.tile_pool(name="xin", bufs=4) as xin,
            tc.tile_pool(name="trig", bufs=2) as trig,
            tc.tile_pool(name="rope", bufs=2) as rope,
            tc.tile_pool(name="probs", bufs=2) as probs,
            tc.tile_pool(name="tree", bufs=2) as treeP,
            tc.tile_pool(name="fold", bufs=2) as fold,
            tc.tile_pool(name="nrm", bufs=1) as nrm,
            tc.tile_pool(name="attn", bufs=2) as attnP,
            tc.tile_pool(name="a2asb", bufs=1) as a2asb,
            tc.tile_pool(name="wosb", bufs=16) as wosb,
            tc.tile_pool(name="oacc", bufs=1) as oacc,
            tc.tile_pool(name="osb", bufs=2) as osb,
        ):
            tensors["qT"] = [
                [
                    persist.tile([128, 512], BF16, tag=f"qT{h}_{sb}", name=f"qT{h}_{sb}")
                    for sb in range(NSB)
                ]
                for h in range(HPC)
            ]
            tensors["kT"] = [
                [
                    persist.tile([128, 512], BF16, tag=f"kT{h}_{sb}", name=f"kT{h}_{sb}")
                    for sb in range(NSB)
                ]
                for h in range(HPC)
            ]
            tensors["v_nat"] = [
                persist.tile([128, 1024], BF16, tag=f"v_nat{sb}", name=f"v_nat{sb}")
                for sb in range(NSB)
            ]
            # weights in SBUF, loaded in chunks on the ACT dma queue
            tensors["w_r"] = {
                wname: persist.tile(
                    [128, NDT * CPC], BF16, tag=f"w{wname}r", name=f"w{wname}r"
                )
                for wname in ("q", "k", "v")
            }

            # statically-zeroed garbage regions of the diagonal prob tiles
            # (allocated once per buf slot; exps never write these columns)
            for buf in range(2):
                pd0 = probs.tile([128, 1024], BF16, tag="pr2d0", name="pr2d0", bufs=2)
                nc.gpsimd.memset(pd0[:, 512:640], 0.0)
                pd1 = probs.tile([128, 1024], BF16, tag="pr2d1", name="pr2d1", bufs=2)
                nc.gpsimd.memset(pd1[:, 0:256], 0.0)
                nc.gpsimd.memset(pd1[:, 512:896], 0.0)

            # PE-side causal triangle for diagonal 128-blocks:
            # ident^T @ trineg adds -1e9 where q-col < key-row
            trineg = persist.tile([128, 128], BF16, tag="trineg", name="trineg")
            nc.any.memset(trineg[:], 0.0)
            nc.gpsimd.affine_select(
                out=trineg[:],
                in_=trineg[:],
                compare_op=mybir.AluOpType.is_ge,
                fill=-1.0e9,
                base=0,
                pattern=[[1, 128]],
                channel_multiplier=-1,
            )
            tensors["trineg"] = trineg
            ident = persist.tile([128, 128], BF16, tag="ident", name="ident")
            nc.any.memset(ident[:], 1.0)
            nc.gpsimd.affine_select(
                out=ident[:],
                in_=ident[:],
                compare_op=mybir.AluOpType.is_ge,
                fill=0.0,
                base=0,
                pattern=[[1, 128]],
                channel_multiplier=-1,
            )
            nc.gpsimd.affine_select(
                out=ident[:],
                in_=ident[:],
                compare_op=mybir.AluOpType.is_ge,
                fill=0.0,
                base=0,
                pattern=[[-1, 128]],
                channel_multiplier=1,
            )
            tensors["ident"] = ident

            a2a_ins = [
                dram.tile([NCORES * 128, 512], BF16, tag=f"a2i{h}", name=f"a2i{h}")
                for h in range(HPC)
            ]
            a2a_outs = [
                dram.tile([NCORES * 128, 512], BF16, tag=f"a2o{h}", name=f"a2o{h}")
                for h in range(HPC)
            ]

            pools = {
                "xin": xin,
                "trig": trig,
                "rope": rope,
                "probs": probs,
                "tree": treeP,
                "fold": fold,
                "nrm": nrm,
                "attn": attnP,
                "a2asb": a2asb,
            }

            def load_w_chunk(g, ndt, only=None):
                # one queue per weight tensor so the streams run in parallel:
                # wq on ACT, wv on gpsimd, wk interleaved on sync (via wk_cb)
                for wname, eng in (
                    ("q", nc.scalar),
                    ("k", nc.sync),
                    ("v", nc.gpsimd),
                ):
                    if only is not None and wname != only:
                        continue
                    wv_ = tensors[f"w{wname}2"].rearrange("(dt p) c -> p dt c", p=128)
                    eng.dma_start(
                        out=tensors["w_r"][wname][
                            :, g * CPC : (g + ndt) * CPC
                        ].rearrange("p (dt c) -> p dt c", dt=ndt),
                        in_=wv_[:, g : g + ndt, :],
                    )

            _wk_sched = {0: (0, 1), 1: (1, 3), 3: (4, 4), 5: (8, 4), 7: (12, 4)}

            def wk_cb(sb, d):
                if sb == 0 and d in _wk_sched:
                    g, ndt = _wk_sched[d]
                    load_w_chunk(g, ndt, only="k")

            tensors["wk_cb"] = wk_cb

            pools["wosb"] = wosb
            pools["oacc"] = oacc
            pools["osb"] = osb
            a2a_r = []
            with tc.tile_pool(name="psQK", bufs=1, space="PSUM") as psQK:
                pools["psQK"] = psQK
                with tc.tile_pool(name="psV", bufs=1, space="PSUM") as psV:
                    pools["psV"] = psV
                    for g, ndt in ((0, 1), (1, 3), (4, 4), (8, 4), (12, 4)):
                        load_w_chunk(g, ndt, only="q")
                        load_w_chunk(g, ndt, only="v")
                    # sb0..sb6 fused qk+v
                    for sb in range(NSB - 1):
                        psb, cosS, sinS = _emit_sb(nc, tensors, pools, sb, "both")
                        _emit_rope_sb(
                            nc, tensors, pools, sb, psb, cosS, sinS, range(HPC)
                        )
                    # sb7 v-pass, then psV closes
                    _emit_sb(nc, tensors, pools, 7, "v")
                # sb7 qk-pass (psQK still open; drains go to DVE)
                psb7, cosS7, sinS7 = _emit_sb(nc, tensors, pools, 7, "qk")
            # psQK released -> psS gets its 4 banks + psV's 2 + 2 spares
            wrs = {}
            with tc.tile_pool(name="psS", bufs=3, space="PSUM") as psS:
                pools["psS"] = psS
                pb = _PhaseB(nc, tensors, pools, a2a_ins, a2a_outs, groups, a2a_r)

                def wo_pre(k2):
                    # 2 wo DMAs on the (mostly idle) gpsimd dma queue; only
                    # h0's 16 tiles prefetch here, so the 16-slot pool never
                    # forces a WAR wait on this queue mid-phase-B
                    for k in (2 * k2, 2 * k2 + 1):
                        if k < 13:
                            dbp, c = divmod(k, NCORES)
                            wrs[(0, dbp, c)] = _emit_wo_dma(
                                nc, tensors, pools, 0, dbp, c, nc.gpsimd
                            )

                # B-early: qb5 ready after sb6; overlaps sb7's drains
                pb.emit_qb(0, 5)
                _emit_rope_sb(nc, tensors, pools, 7, psb7, cosS7, sinS7, [0])
                wo_pre(0)
                pb.emit_qb(0, 6)
                wo_pre(1)
                pb.emit_qb(0, 7)
                wo_pre(2)
                pb.emit_qb(0, 1)
                _emit_rope_sb(nc, tensors, pools, 7, psb7, cosS7, sinS7, [1])
                wo_pre(3)
                for i, qb in enumerate((2, 3, 4, 0)):
                    pb.emit_qb(0, qb)
                    wo_pre(4 + i)
                for i, qb in enumerate((1, 2, 3, 4, 5, 6, 7, 0)):
                    pb.emit_qb(1, qb)
                    wo_pre(8 + i)
                pb.flush1()
                pb.flush2()
            with tc.tile_pool(name="psC", bufs=1, space="PSUM") as psC:
                pools["psC"] = psC
                _phase_c(nc, tensors, pools, a2a_r, out_part, wrs)

    nc.compile()
    return nc


_NC_CACHE = None


def _get_program():
    global _NC_CACHE
    if _NC_CACHE is None:
        _NC_CACHE = _build_program()
    return _NC_CACHE


def _rope_tables():
    # match reference's f32 arithmetic
    i = np.arange(0, HD, 2, dtype=np.float32) / np.float32(HD)
    freqs = (np.float32(1.0) / np.float32(ROPE_THETA) ** i).astype(np.float32)  # [64]
    ang = np.arange(S, dtype=np.float32)[:, None] * freqs[None, :]  # [S, 64]
    cos = np.cos(ang).astype(np.float32).T  # [64, S]
    sin = np.sin(ang).astype(np.float32).T
    cosf = np.concatenate([cos, cos], axis=0)  # [128, S]
    # pre-swapped for the partition-offset rope muls: rows [0:64] multiply
    # p[64:128] reading sinS[64:128] = -sin, rows [64:128] read [0:64] = +sin
    sinf = np.concatenate([sin, -sin], axis=0)
    return np.ascontiguousarray(cosf), np.ascontiguousarray(sinf)


def kernel(x, mask, wq, wk, wv, wo):
    # mask is the standard causal mask produced by setup_inputs; causality is
    # implemented directly in the device program, so the tensor itself is not
    # shipped to the cores.
    import ml_dtypes

    x = np.asarray(x, dtype=np.float32)
    wq = np.asarray(wq, dtype=np.float32)
    wk = np.asarray(wk, dtype=np.float32)
    wv = np.asarray(wv, dtype=np.float32)
    wo_b = np.ascontiguousarray(np.asarray(wo, dtype=np.float32).astype(ml_dtypes.bfloat16))

    xT = np.ascontiguousarray(
        x.reshape(S, D).T.astype(ml_dtypes.bfloat16)
    )  # [D, S] bf16

    # de-interleave permutation within each head (RoPE pairs -> halves)
    idx = np.concatenate([np.arange(0, HD, 2), np.arange(1, HD, 2)])
    perm = np.concatenate([h * HD + idx for h in range(H)])
    wq_p = wq[:, perm]
    wk_p = wk[:, perm]

    cosf, sinf = _rope_tables()

    nc = _get_program()
    in_maps = []
    for c in range(NCORES):
        csl = slice(c * CPC, (c + 1) * CPC)
        in_maps.append(
            {
                "xT": xT,
                "wq2": np.ascontiguousarray(wq_p[:, csl].astype(ml_dtypes.bfloat16)),
                "wk2": np.ascontiguousarray(wk_p[:, csl].astype(ml_dtypes.bfloat16)),
                "wv2": np.ascontiguousarray(wv[:, csl].astype(ml_dtypes.bfloat16)),
                "wo_full": wo_b,
                "cosf": cosf,
                "sinf": sinf,
            }
        )
    res = run_bass_kernel_spmd(nc, in_maps, core_ids=list(range(NCORES)))
    out = np.concatenate([res.results[c]["out_part"] for c in range(NCORES)], axis=0)
    return out.reshape(1, S, D).astype(np.float32)


# revision 34
# speedup vs baseline: 1.2320x; 1.1276x over previous
"""Causal multi-head attention (B=1, S=4096, D=2048, H=16) on 8 trn2 cores.

Sharding: tensor-parallel over heads (2 heads/core) for QKV + attention;
output projection is head-sharded (row-parallel wo) with a per-head
AllToAll over sequence rows; the host concatenates the 8 row-slices.

v3 design notes (PE sustained clock is externally throttled to 13/16 =
1.95GHz; exec time ~= matmul-moving-columns/1.95GHz + stalls, so the
design minimizes columns and keeps every engine queue short):
 - causal 128-granularity on the diagonal 512-blocks: score/PV matmuls and
   exps are width-restricted; diagonal prob tiles have statically-zeroed
   garbage regions (memset once) so no mask tensors are needed at all.
 - softmax denominator: pairwise DVE tree over [128,1024] bf16 prob tiles,
   then gpsimd partition_all_reduce (3.5us, idle engine) + DVE reciprocal.
   No ones-matmul, no broadcast matmul -- zero PE cycles.
 - RoPE entirely on DVE via partition-offset muls (sin table is [-sin;sin]
   stacked), no ACT rot-copies: ACT runs only psum drains + exps, since
   phase-B exp throughput (~1.0us per [128,1024]) is at parity with PE.
 - PSUM is bank-granular: A uses psQK(4)+psV(2); sb7 runs v-pass then
   qk-pass so pools close early and phase B's ss2 (bufs=3) starts with no
   write-after-read stall; B overlaps A's tail via qb5/qb6 (ready after
   sb6).  Interleaved j-chunks share a V bank via start-once accumulation.
 - phase C is ss-serial (8-matmul groups, 4-bank double buffering), head-
   split so head-0's 33us of matmuls cover head-1's AllToAll; wo rows are
   prefetched on the ACT dma queue during phase B.
"""

import sys

for _p in ("/opt/trn_rl_repo", "/root/.axon_site/_ro/trn_rl_repo"):
    if _p not in sys.path:
        sys.path.insert(0, _p)

import numpy as np

import concourse.bacc as bacc
import concourse.mybir as mybir
from concourse import bass_isa
from concourse.bass_utils import run_bass_kernel_spmd
from concourse.tile import TileContext

F32 = mybir.dt.float32
BF16 = mybir.dt.bfloat16

S = 4096
D = 2048
H = 16
HD = 128
NCORES = 8
HPC = H // NCORES  # heads per core = 2
CPC = HPC * HD  # cols per core = 256
ROPE_THETA = 10000.0
SCALE = 1.0 / float(np.sqrt(np.float32(HD)))

NSB = S // 512  # 8 seq blocks of 512
NDT = D // 128  # 16 contraction tiles


def _rope_pair(nc, dst, p, cosS, sinS, pool):
    """dst = p*cos + rot(p)*sin, rot swaps partition halves; sinS rows are
    [sin; -sin] (pre-swapped) so each offset mul reads both SBUF inputs at
    the SAME base partition (BIR constraint) while the output is offset.
    4 DVE ops, no ACT."""
    t1 = pool.tile([128, 512], F32, tag="t1", name="t1")
    nc.vector.tensor_mul(t1[:], p[:], cosS[:])
    t2 = pool.tile([128, 512], F32, tag="t2", name="t2")
    nc.vector.tensor_mul(t2[0:64, :], p[64:128, :], sinS[64:128, :])
    nc.vector.tensor_mul(t2[64:128, :], p[0:64, :], sinS[0:64, :])
    nc.vector.tensor_add(dst[:], t1[:], t2[:])


def _emit_sb(nc, tensors, pools, sb, part):
    """Phase A for one seq block.  part: 'both' | 'v' | 'qk' (sb7 split).
    Weights are already streaming into w_r; xr tiles come from the xin pool.
    qk psum -> psb tiles (drained on DVE for sb7, ACT otherwise) + RoPE on
    DVE; v psum -> v_nat via ACT."""
    xT = tensors["xT"]
    cosf, sinf = tensors["cosf"], tensors["sinf"]
    w_r = tensors["w_r"]
    qT, kT, v_nat = tensors["qT"], tensors["kT"], tensors["v_nat"]
    xin, trig, rope, psQK, psV = (
        pools["xin"],
        pools["trig"],
        pools["rope"],
        pools.get("psQK"),
        pools.get("psV"),
    )
    sl = slice(sb * 512, (sb + 1) * 512)

    do_qk = part in ("both", "qk")
    do_v = part in ("both", "v")

    if do_qk:
        cosS = trig.tile([128, 512], F32, tag="cosS", name="cosS")
        sinS = trig.tile([128, 512], F32, tag="sinS", name="sinS")
        if sb > 0:  # sb0's tables load after its xr stream (startup path)
            nc.sync.dma_start(out=cosS[:], in_=cosf[:, sl])
            nc.sync.dma_start(out=sinS[:], in_=sinf[:, sl])
        ps = {
            t: psQK.tile([128, 512], F32, tag=f"ps_{t}", name=f"ps_{t}")
            for t in ("q0", "q1", "k0", "k1")
        }
    if do_v:
        vps = [
            psV.tile([128, 512], F32, tag=f"ps_v{j2}", name=f"ps_v{j2}")
            for j2 in range(2)
        ]

    for d in range(NDT):
        xr = xin.tile([128, 512], BF16, tag="xr", name="xr")
        nc.sync.dma_start(out=xr[:], in_=xT[d * 128 : (d + 1) * 128, sl])
        wk_cb = tensors.get("wk_cb")
        if wk_cb is not None and part != "v":
            wk_cb(sb, d)
        st = dict(start=(d == 0), stop=(d == NDT - 1))
        if do_qk:
            for h in range(HPC):
                co = d * CPC + h * HD
                nc.tensor.matmul(
                    ps[f"q{h}"][:], w_r["q"][:, co : co + HD], xr[:], **st
                )
                nc.tensor.matmul(
                    ps[f"k{h}"][:], w_r["k"][:, co : co + HD], xr[:], **st
                )
        if do_v:
            for j in range(4):
                # two j-chunks share a psum bank: only the bank's first
                # matmul clears has_written (start-once), the second chunk's
                # d==0 matmul lands on clear bits and overwrites per-element
                nc.tensor.matmul(
                    vps[j // 2][:, (j % 2) * 256 : (j % 2) * 256 + 256],
                    xr[:, j * 128 : (j + 1) * 128],
                    w_r["v"][:, d * CPC : (d + 1) * CPC],
                    start=(d == 0 and j % 2 == 0),
                    stop=(d == NDT - 1),
                    skip_group_check=True,
                )

    if do_qk and sb == 0:
        nc.sync.dma_start(out=cosS[:], in_=cosf[:, sl])
        nc.sync.dma_start(out=sinS[:], in_=sinf[:, sl])
    if do_v:
        for j2 in range(2):
            nc.scalar.copy(v_nat[sb][:, j2 * 512 : (j2 + 1) * 512], vps[j2][:])
    if do_qk:
        psb = {}
        for h in range(HPC):
            for t in (f"q{h}", f"k{h}"):
                pt = rope.tile([128, 512], F32, tag=f"psb_{t}", name=f"psb_{t}", bufs=1)
                if part == "qk":  # sb7: ACT is needed for phase-B exps
                    nc.vector.tensor_copy(pt[:], ps[t][:])
                else:
                    nc.scalar.copy(pt[:], ps[t][:])
                psb[t] = pt
        return psb, cosS, sinS
    return None, None, None


def _emit_rope_sb(nc, tensors, pools, sb, psb, cosS, sinS, heads):
    qT, kT = tensors["qT"], tensors["kT"]
    rope = pools["rope"]
    for h in heads:
        _rope_pair(nc, qT[h][sb], psb[f"q{h}"], cosS, sinS, rope)
        _rope_pair(nc, kT[h][sb], psb[f"k{h}"], cosS, sinS, rope)


class _PhaseB:
    """Per-(head, q-block) causal attention with 2-group exp lookahead,
    width-restricted diagonal, pairwise denominator tree, gpsimd all-reduce
    normalization, and deferred flush for a2a overlap."""

    def __init__(self, nc, tensors, pools, a2a_ins, a2a_outs, groups, a2a_r):
        self.nc = nc
        self.t = tensors
        self.p = pools
        self.a2a_ins = a2a_ins
        self.a2a_outs = a2a_outs
        self.groups = groups
        self.a2a_r = a2a_r
        self.pending = None  # awaiting flush1 (all-reduce kickoff)
        self.pending2 = None  # awaiting flush2 (recip/at/dma)

    def flush1(self):
        """Kick the gpsimd all-reduce for the previous qb.  Only gpsimd-queue
        ops here -- nothing that could head-of-line block DVE."""
        nc = self.nc
        if self.pending is None:
            return
        fh, fqb, fpo, fprS = self.pending
        self.pending = None
        nrm = self.p["nrm"]
        denf = nrm.tile([128, 512], F32, tag="denf", name="denf")
        nc.gpsimd.partition_all_reduce(
            denf[:], fprS[:], channels=128, reduce_op=bass_isa.ReduceOp.add
        )
        self.pending2 = (fh, fqb, fpo, denf)

    def flush2(self):
        """DVE tail of the flush, emitted ~4 groups after flush1 so the
        reciprocal never waits on the all-reduce in the DVE FIFO."""
        nc = self.nc
        if self.pending2 is None:
            return
        fh, fqb, fpo, denf = self.pending2
        self.pending2 = None
        nrm, attnP, a2asb = self.p["nrm"], self.p["attn"], self.p["a2asb"]
        rcsf = nrm.tile([128, 512], F32, tag="rcsf", name="rcsf")
        nc.vector.reciprocal_approx_fast(out=rcsf[:], in_=denf[:])
        at = attnP.tile([128, 512], BF16, tag="at", name="at")
        nc.vector.tensor_mul(at[:], fpo[:], rcsf[:])
        # gpsimd dma queue: never behind collective-gated transfers (the
        # sync queue carries the ar staging, which waits on the AllToAll)
        nc.gpsimd.dma_start(
            out=self.a2a_ins[fh][fqb * 128 : (fqb + 1) * 128, :], in_=at[:]
        )
        if fqb == 0:  # qb0 is processed last per head
            nc.gpsimd.collective_compute(
                "AllToAll",
                mybir.AluOpType.bypass,
                replica_groups=self.groups,
                ins=[self.a2a_ins[fh].opt()],
                outs=[self.a2a_outs[fh].opt()],
            )
            ar = a2asb.tile(
                [128, NCORES * 512], BF16, tag=f"a2r{fh}", name=f"a2r{fh}"
            )
            for c in range(NCORES):
                nc.sync.dma_start(
                    out=ar[:, c * 512 : (c + 1) * 512],
                    in_=self.a2a_outs[fh][c * 128 : (c + 1) * 128, :],
                )
            self.a2a_r.append(ar)

    @staticmethod
    def _kt_off(qb, g, half):
        kt = 2 * g + half
        if kt >= 4 * qb:  # diagonal 512-block
            return (kt - 4 * qb) * 128
        return 0

    def _emit_sc(self, item):
        nc = self.nc
        h, qb, g, G = item
        kT, qT = self.t["kT"], self.t["qT"]
        ss2 = self.p["psS"].tile([128, 1024], F32, tag="ss2", name="ss2")
        self.ss2s[(h, qb, g)] = ss2
        for half in range(2):
            kt = 2 * g + half
            off = self._kt_off(qb, g, half)
            diag = kt >= 4 * qb
            nc.tensor.matmul(
                ss2[:, half * 512 + off : (half + 1) * 512],
                kT[h][kt // 4][:, (kt % 4) * 128 : (kt % 4 + 1) * 128],
                qT[h][qb][:, off:512],
                start=True,
                stop=not diag,
                skip_group_check=diag,
            )
            if diag:
                # accumulate -1e9 into the strictly-upper triangle of the
                # first 128 q-cols (PE-side causal mask; exp -> exact 0)
                nc.tensor.matmul(
                    ss2[:, half * 512 + off : half * 512 + off + 128],
                    self.t["ident"][:],
                    self.t["trineg"][:],
                    start=False,
                    stop=True,
                    skip_group_check=True,
                )

    def _emit_exp(self, item):
        nc = self.nc
        h, qb, g, G = item
        probs = self.p["probs"]
        ss2 = self.ss2s.pop((h, qb, g))
        diag = self._kt_off(qb, g, 1) > 0
        if not diag:
            pr2 = probs.tile([128, 1024], BF16, tag="pr2", name="pr2", bufs=4)
            nc.scalar.activation(
                pr2[:], ss2[:], mybir.ActivationFunctionType.Exp, scale=SCALE
            )
        else:
            dt_tag = "pr2d0" if self._kt_off(qb, g, 0) == 0 else "pr2d1"
            pr2 = probs.tile([128, 1024], BF16, tag=dt_tag, name=dt_tag, bufs=2)
            for half in range(2):
                off = self._kt_off(qb, g, half)
                nc.scalar.activation(
                    pr2[:, half * 512 + off : (half + 1) * 512],
                    ss2[:, half * 512 + off : (half + 1) * 512],
                    mybir.ActivationFunctionType.Exp,
                    scale=SCALE,
                )
        self.pr2s[(h, qb, g)] = pr2

    def _emit_pv(self, item):
        nc = self.nc
        h, qb, g, G = item
        v_nat = self.t["v_nat"]
        if g == 0:
            self.po[(h, qb)] = self.p["psS"].tile(
                [128, 512], F32, tag="po", name="po", bufs=2
            )
        po = self.po[(h, qb)]
        pr2 = self.pr2s[(h, qb, g)]
        for half in range(2):
            kt = 2 * g + half
            off = self._kt_off(qb, g, half)
            nc.tensor.matmul(
                po[:, off:512],
                v_nat[kt // 4][
                    :, ((kt % 4) * 2 + h) * 128 : ((kt % 4) * 2 + h + 1) * 128
                ],
                pr2[:, half * 512 + off : (half + 1) * 512],
                start=(g == 0 and half == 0),
                stop=(g == G - 1 and half == 1),
                skip_group_check=True,
            )

    def _tree_push(self, item):
        nc = self.nc
        h, qb, g, G = item
        treeP = self.p["tree"]
        if g % 2 == 1:
            tree = self.tree.setdefault((h, qb), [])
            s = treeP.tile([128, 1024], BF16, tag="tl0", name="tl0", bufs=2)
            nc.vector.tensor_add(
                s[:],
                self.pr2s.pop((h, qb, g - 1))[:],
                self.pr2s.pop((h, qb, g))[:],
            )
            tree.append((1, s))
            while len(tree) >= 2 and tree[-1][0] == tree[-2][0]:
                l1, t1 = tree.pop()
                _, t2 = tree.pop()
                tn = treeP.tile(
                    [128, 1024], BF16, tag=f"tl{l1}", name=f"tl{l1}", bufs=2
                )
                nc.vector.tensor_add(tn[:], t1[:], t2[:])
                tree.append((l1 + 1, tn))
        if g == G - 1:  # finalize qb: collapse + fold, hand off to flush1
            tree = self.tree.pop((h, qb))
            while len(tree) > 1:
                l1, t1 = tree.pop()
                _, t2 = tree.pop()
                tn = treeP.tile([128, 1024], BF16, tag="tlc", name="tlc", bufs=2)
                nc.vector.tensor_add(tn[:], t1[:], t2[:])
                tree.append((l1 + 1, tn))
            root = tree[0][1]
            prS = self.p["fold"].tile([128, 512], BF16, tag="prS", name="prS")
            nc.vector.tensor_add(prS[:], root[:, 0:512], root[:, 512:1024])
            self.pending = (h, qb, self.po.pop((h, qb)), prS)

    def run(self, entries):
        """entries: (h, qb) tuples and callables (side work for other engine
        queues).  One continuous group pipeline -- the score/exp lookahead
        crosses qb boundaries so the PE/ACT pipeline never resets."""
        self.ss2s, self.pr2s, self.po, self.tree = {}, {}, {}, {}
        stream = []
        for e in entries:
            if callable(e):
                stream.append(e)
            else:
                h, qb = e
                G = 2 * qb + 2
                stream.extend((h, qb, g, G) for g in range(G))
        glist = [x for x in stream if not callable(x)]
        self._emit_sc(glist[0])
        self._emit_exp(glist[0])
        if len(glist) > 1:
            self._emit_sc(glist[1])
            self._emit_exp(glist[1])
        k = 0
        for item in stream:
            if callable(item):
                item()
                continue
            h, qb, g, G = item
            if g == 1:
                self.flush1()
            if g == min(5, G - 1):
                self.flush2()
            if k + 2 < len(glist):
                self._emit_sc(glist[k + 2])
            self._emit_pv(item)
            self._tree_push(item)
            if k + 2 < len(glist):
                self._emit_exp(glist[k + 2])
            k += 1
        self.flush1()
        self.flush2()


def _emit_wo_dma(nc, tensors, pools, h, dbp, c, queue):
    """One wo row-block DMA into the wosb pool (16 slots; h0's 16 tiles are
    prefetched on the gpsimd queue during phase B, h1's on ACT during C)."""
    wosb = pools["wosb"]
    wo_full = tensors["wo_full"]
    wr = wosb.tile([128, 1024], BF16, tag="wr", name="wr", bufs=13)
    ro = c * CPC + h * HD
    queue.dma_start(
        out=wr[:], in_=wo_full[ro : ro + 128, dbp * 1024 : (dbp + 1) * 1024]
    )
    return wr


def _phase_c(nc, tensors, pools, a2a_r, out_part, wrs):
    """Out-projection, ss-serial: per (h, dbp, j, ss) an 8-matmul psum group
    over source cores; h0 drains to bf16 SBUF acc, h1 adds + streams out."""
    psC, oacc, osb = (
        pools["psC"],
        pools["oacc"],
        pools["osb"],
    )
    # remaining wo rows stream in on the ACT dma queue while h0 computes
    for h in range(HPC):
        for dbp in range(2):
            for c in range(NCORES):
                if (h, dbp, c) not in wrs:
                    wrs[(h, dbp, c)] = _emit_wo_dma(
                        nc, tensors, pools, h, dbp, c, nc.scalar
                    )

    acc = {}
    for h in range(HPC):
        for dbp in range(2):
            for j in range(2):
                db = dbp * 2 + j
                for ss in range(4):
                    pc = psC.tile([128, 512], F32, tag="pc", name="pc", bufs=4)
                    for c in range(NCORES):
                        nc.tensor.matmul(
                            pc[:],
                            a2a_r[h][
                                :, c * 512 + ss * 128 : c * 512 + (ss + 1) * 128
                            ],
                            wrs[(h, dbp, c)][:, j * 512 : (j + 1) * 512],
                            start=(c == 0),
                            stop=(c == NCORES - 1),
                        )
                    if h == 0:
                        a = oacc.tile(
                            [128, 512], BF16, tag=f"acc{ss}{db}", name=f"acc{ss}{db}"
                        )
                        nc.scalar.copy(a[:], pc[:])
                        acc[(ss, db)] = a
                    else:
                        oc = osb.tile([128, 512], F32, tag="oc", name="oc")
                        nc.vector.tensor_add(oc[:], pc[:], acc[(ss, db)][:])
                        nc.sync.dma_start(
                            out=out_part[
                                ss * 128 : (ss + 1) * 128, db * 512 : (db + 1) * 512
                            ],
                            in_=oc[:],
                        )


def _build_program():
    nc = bacc.Bacc("TRN2", target_bir_lowering=False)

    tensors = {}
    tensors["xT"] = nc.dram_tensor("xT", [D, S], BF16, kind="ExternalInput")
    tensors["wq2"] = nc.dram_tensor("wq2", [D, CPC], BF16, kind="ExternalInput")
    tensors["wk2"] = nc.dram_tensor("wk2", [D, CPC], BF16, kind="ExternalInput")
    tensors["wv2"] = nc.dram_tensor("wv2", [D, CPC], BF16, kind="ExternalInput")
    tensors["wo_full"] = nc.dram_tensor("wo_full", [D, D], BF16, kind="ExternalInput")
    tensors["cosf"] = nc.dram_tensor("cosf", [HD, S], F32, kind="ExternalInput")
    tensors["sinf"] = nc.dram_tensor("sinf", [HD, S], F32, kind="ExternalInput")
    out_part = nc.dram_tensor("out_part", [S // NCORES, D], F32, kind="ExternalOutput")

    groups = [list(range(NCORES))]

    with TileContext(nc) as tc:
        with (
            tc.tile_pool(name="persist", bufs=1) as persist,
            tc.tile_pool(name="dram", bufs=1, space="DRAM") as dram,
            tc.tile_pool(name="xin", bufs=4) as xin,
            tc.tile_pool(name="trig", bufs=2) as trig,
            tc.tile_pool(name="rope", bufs=2) as rope,
            tc.tile_pool(name="probs", bufs=2) as probs,
            tc.tile_pool(name="tree", bufs=2) as treeP,
            tc.tile_pool(name="fold", bufs=2) as fold,
            tc.tile_pool(name="nrm", bufs=1) as nrm,
            tc.tile_pool(name="attn", bufs=2) as attnP,
            tc.tile_pool(name="a2asb", bufs=1) as a2asb,
            tc.tile_pool(name="wosb", bufs=16) as wosb,
            tc.tile_pool(name="oacc", bufs=1) as oacc,
            tc.tile_pool(name="osb", bufs=2) as osb,
        ):
            tensors["qT"] = [
                [
                    persist.tile([128, 512], BF16, tag=f"qT{h}_{sb}", name=f"qT{h}_{sb}")
                    for sb in range(NSB)
                ]
                for h in range(HPC)
            ]
            tensors["kT"] = [
                [
                    persist.tile([128, 512], BF16, tag=f"kT{h}_{sb}", name=f"kT{h}_{sb}")
                    for sb in range(NSB)
                ]
                for h in range(HPC)
            ]
            tensors["v_nat"] = [
                persist.tile([128, 1024], BF16, tag=f"v_nat{sb}", name=f"v_nat{sb}")
                for sb in range(NSB)
            ]
            # weights in SBUF, loaded in chunks on the ACT dma queue
            tensors["w_r"] = {
                wname: persist.tile(
                    [128, NDT * CPC], BF16, tag=f"w{wname}r", name=f"w{wname}r"
                )
                for wname in ("q", "k", "v")
            }

            # statically-zeroed garbage regions of the diagonal prob tiles
            # (allocated once per buf slot; exps never write these columns)
            for buf in range(2):
                pd0 = probs.tile([128, 1024], BF16, tag="pr2d0", name="pr2d0", bufs=2)
                nc.gpsimd.memset(pd0[:, 512:640], 0.0)
                pd1 = probs.tile([128, 1024], BF16, tag="pr2d1", name="pr2d1", bufs=2)
                nc.gpsimd.memset(pd1[:, 0:256], 0.0)
                nc.gpsimd.memset(pd1[:, 512:896], 0.0)

            # PE-side causal triangle for diagonal 128-blocks:
            # ident^T @ trineg adds -1e9 where q-col < key-row
            trineg = persist.tile([128, 128], BF16, tag="trineg", name="trineg")
            nc.any.memset(trineg[:], 0.0)
            nc.gpsimd.affine_select(
                out=trineg[:],
                in_=trineg[:],
                compare_op=mybir.AluOpType.is_ge,
                fill=-1.0e9,
                base=0,
                pattern=[[1, 128]],
                channel_multiplier=-1,
            )
            tensors["trineg"] = trineg
            ident = persist.tile([128, 128], BF16, tag="ident", name="ident")
            nc.any.memset(ident[:], 1.0)
            nc.gpsimd.affine_select(
                out=ident[:],
                in_=ident[:],
                compare_op=mybir.AluOpType.is_ge,
                fill=0.0,
                base=0,
                pattern=[[1, 128]],
                channel_multiplier=-1,
            )
            nc.gpsimd.affine_select(
                out=ident[:],
                in_=ident[:],
                compare_op=mybir.AluOpType.is_ge,
                fill=0.0,
                base=0,
                pattern=[[-1, 128]],
                channel_multiplier=1,
            )
            tensors["ident"] = ident

            a2a_ins = [
                dram.tile([NCORES * 128, 512], BF16, tag=f"a2i{h}", name=f"a2i{h}")
                for h in range(HPC)
            ]
            a2a_outs = [
                dram.tile([NCORES * 128, 512], BF16, tag=f"a2o{h}", name=f"a2o{h}")
                for h in range(HPC)
            ]

            pools = {
                "xin": xin,
                "trig": trig,
                "rope": rope,
                "probs": probs,
                "tree": treeP,
                "fold": fold,
                "nrm": nrm,
                "attn": attnP,
                "a2asb": a2asb,
            }

            def load_w_chunk(g, ndt, only=None):
                # one queue per weight tensor so the streams run in parallel:
                # wq on ACT, wv on gpsimd, wk interleaved on sync (via wk_cb)
                for wname, eng in (
                    ("q", nc.scalar),
                    ("k", nc.sync),
                    ("v", nc.gpsimd),
                ):
                    if only is not None and wname != only:
                        continue
                    wv_ = tensors[f"w{wname}2"].rearrange("(dt p) c -> p dt c", p=128)
                    eng.dma_start(
                        out=tensors["w_r"][wname][
                            :, g * CPC : (g + ndt) * CPC
                        ].rearrange("p (dt c) -> p dt c", dt=ndt),
                        in_=wv_[:, g : g + ndt, :],
                    )

            _wk_sched = {0: (0, 1), 1: (1, 3), 3: (4, 4), 5: (8, 4), 7: (12, 4)}

            def wk_cb(sb, d):
                if sb == 0 and d in _wk_sched:
                    g, ndt = _wk_sched[d]
                    load_w_chunk(g, ndt, only="k")

            tensors["wk_cb"] = wk_cb

            pools["wosb"] = wosb
            pools["oacc"] = oacc
            pools["osb"] = osb
            a2a_r = []
            with tc.tile_pool(name="psQK", bufs=1, space="PSUM") as psQK:
                pools["psQK"] = psQK
                with tc.tile_pool(name="psV", bufs=1, space="PSUM") as psV:
                    pools["psV"] = psV
                    for g, ndt in ((0, 1), (1, 3), (4, 4), (8, 4), (12, 4)):
                        load_w_chunk(g, ndt, only="q")
                        load_w_chunk(g, ndt, only="v")
                    # sb0..sb6 fused qk+v
                    for sb in range(NSB - 1):
                        psb, cosS, sinS = _emit_sb(nc, tensors, pools, sb, "both")
                        _emit_rope_sb(
                            nc, tensors, pools, sb, psb, cosS, sinS, range(HPC)
                        )
                    # sb7 v-pass, then psV closes
                    _emit_sb(nc, tensors, pools, 7, "v")
                # sb7 qk-pass (psQK still open; drains go to DVE)
                psb7, cosS7, sinS7 = _emit_sb(nc, tensors, pools, 7, "qk")
            # psQK released -> psS gets its 4 banks + psV's 2 + 2 spares
            wrs = {}
            with tc.tile_pool(name="psS", bufs=3, space="PSUM") as psS:
                pools["psS"] = psS
                pb = _PhaseB(nc, tensors, pools, a2a_ins, a2a_outs, groups, a2a_r)

                def wo_pre(k2):
                    # 2 wo DMAs on the (mostly idle) gpsimd dma queue; only
                    # h0's 16 tiles prefetch here, so the 16-slot pool never
                    # forces a WAR wait on this queue mid-phase-B
                    for k in (2 * k2, 2 * k2 + 1):
                        if k < 13:
                            dbp, c = divmod(k, NCORES)
                            wrs[(0, dbp, c)] = _emit_wo_dma(
                                nc, tensors, pools, 0, dbp, c, nc.gpsimd
                            )

                # B-early: qb5 ready after sb6; overlaps sb7's drains
                entries = [
                    (0, 5),
                    lambda: _emit_rope_sb(
                        nc, tensors, pools, 7, psb7, cosS7, sinS7, [0]
                    ),
                    lambda: wo_pre(0),
                    (0, 6),
                    lambda: wo_pre(1),
                    (0, 7),
                    lambda: wo_pre(2),
                    (0, 1),
                    lambda: _emit_rope_sb(
                        nc, tensors, pools, 7, psb7, cosS7, sinS7, [1]
                    ),
                    lambda: wo_pre(3),
                ]
                for i, qb in enumerate((2, 3, 4, 0)):
                    entries.append((0, qb))
                    entries.append(lambda i=i: wo_pre(4 + i))
                for i, qb in enumerate((1, 2, 3, 4, 5, 6, 7, 0)):
                    entries.append((1, qb))
                    entries.append(lambda i=i: wo_pre(8 + i))
                pb.run(entries)
            with tc.tile_pool(name="psC", bufs=1, space="PSUM") as psC:
                pools["psC"] = psC
                _phase_c(nc, tensors, pools, a2a_r, out_part, wrs)

    nc.compile()
    return nc


_NC_CACHE = None


def _get_program():
    global _NC_CACHE
    if _NC_CACHE is None:
        _NC_CACHE = _build_program()
    return _NC_CACHE


def _rope_tables():
    # match reference's f32 arithmetic
    i = np.arange(0, HD, 2, dtype=np.float32) / np.float32(HD)
    freqs = (np.float32(1.0) / np.float32(ROPE_THETA) ** i).astype(np.float32)  # [64]
    ang = np.arange(S, dtype=np.float32)[:, None] * freqs[None, :]  # [S, 64]
    cos = np.cos(ang).astype(np.float32).T  # [64, S]
    sin = np.sin(ang).astype(np.float32).T
    cosf = np.concatenate([cos, cos], axis=0)  # [128, S]
    # pre-swapped for the partition-offset rope muls: rows [0:64] multiply
    # p[64:128] reading sinS[64:128] = -sin, rows [64:128] read [0:64] = +sin
    sinf = np.concatenate([sin, -sin], axis=0)
    return np.ascontiguousarray(cosf), np.ascontiguousarray(sinf)


def kernel(x, mask, wq, wk, wv, wo):
    # mask is the standard causal mask produced by setup_inputs; causality is
    # implemented directly in the device program, so the tensor itself is not
    # shipped to the cores.
    import ml_dtypes

    x = np.asarray(x, dtype=np.float32)
    wq = np.asarray(wq, dtype=np.float32)
    wk = np.asarray(wk, dtype=np.float32)
    wv = np.asarray(wv, dtype=np.float32)
    wo_b = np.ascontiguousarray(np.asarray(wo, dtype=np.float32).astype(ml_dtypes.bfloat16))

    xT = np.ascontiguousarray(
        x.reshape(S, D).T.astype(ml_dtypes.bfloat16)
    )  # [D, S] bf16

    # de-interleave permutation within each head (RoPE pairs -> halves)
    idx = np.concatenate([np.arange(0, HD, 2), np.arange(1, HD, 2)])
    perm = np.concatenate([h * HD + idx for h in range(H)])
    wq_p = wq[:, perm]
    wk_p = wk[:, perm]

    cosf, sinf = _rope_tables()

    nc = _get_program()
    in_maps = []
    for c in range(NCORES):
        csl = slice(c * CPC, (c + 1) * CPC)
        in_maps.append(
            {
                "xT": xT,
                "wq2": np.ascontiguousarray(wq_p[:, csl].astype(ml_dtypes.bfloat16)),
                "wk2": np.ascontiguousarray(wk_p[:, csl].astype(ml_dtypes.bfloat16)),
                "wv2": np.ascontiguousarray(wv[:, csl].astype(ml_dtypes.bfloat16)),
                "wo_full": wo_b,
                "cosf": cosf,
                "sinf": sinf,
            }
        )
    res = run_bass_kernel_spmd(nc, in_maps, core_ids=list(range(NCORES)))
    out = np.concatenate([res.results[c]["out_part"] for c in range(NCORES)], axis=0)
    return out.reshape(1, S, D).astype(np.float32)


# revision 37
# speedup vs baseline: 1.2630x; 1.0252x over previous
"""Causal multi-head attention (B=1, S=4096, D=2048, H=16) on 8 trn2 cores.

Sharding: tensor-parallel over heads (2 heads/core) for QKV + attention;
output projection is head-sharded (row-parallel wo) with a per-head
AllToAll over sequence rows; the host concatenates the 8 row-slices.

v3 design notes (PE sustained clock is externally throttled to 13/16 =
1.95GHz; exec time ~= matmul-moving-columns/1.95GHz + stalls, so the
design minimizes columns and keeps every engine queue short):
 - causal 128-granularity on the diagonal 512-blocks: score/PV matmuls and
   exps are width-restricted; diagonal prob tiles have statically-zeroed
   garbage regions (memset once) so no mask tensors are needed at all.
 - softmax denominator: pairwise DVE tree over [128,1024] bf16 prob tiles,
   then gpsimd partition_all_reduce (3.5us, idle engine) + DVE reciprocal.
   No ones-matmul, no broadcast matmul -- zero PE cycles.
 - RoPE entirely on DVE via partition-offset muls (sin table is [-sin;sin]
   stacked), no ACT rot-copies: ACT runs only psum drains + exps, since
   phase-B exp throughput (~1.0us per [128,1024]) is at parity with PE.
 - PSUM is bank-granular: A uses psQK(4)+psV(2); sb7 runs v-pass then
   qk-pass so pools close early and phase B's ss2 (bufs=3) starts with no
   write-after-read stall; B overlaps A's tail via qb5/qb6 (ready after
   sb6).  Interleaved j-chunks share a V bank via start-once accumulation.
 - phase C is ss-serial (8-matmul groups, 4-bank double buffering), head-
   split so head-0's 33us of matmuls cover head-1's AllToAll; wo rows are
   prefetched on the ACT dma queue during phase B.
"""

import sys

for _p in ("/opt/trn_rl_repo", "/root/.axon_site/_ro/trn_rl_repo"):
    if _p not in sys.path:
        sys.path.insert(0, _p)

import numpy as np

import concourse.bacc as bacc
import concourse.mybir as mybir
from concourse import bass_isa
from concourse.bass_utils import run_bass_kernel_spmd
from concourse.tile import TileContext

F32 = mybir.dt.float32
BF16 = mybir.dt.bfloat16

S = 4096
D = 2048
H = 16
HD = 128
NCORES = 8
HPC = H // NCORES  # heads per core = 2
CPC = HPC * HD  # cols per core = 256
ROPE_THETA = 10000.0
SCALE = 1.0 / float(np.sqrt(np.float32(HD)))

NSB = S // 512  # 8 seq blocks of 512
NDT = D // 128  # 16 contraction tiles


def _rope_pair(nc, dst, p, cosS, sinS, pool):
    """dst = p*cos + rot(p)*sin, rot swaps partition halves; sinS rows are
    [sin; -sin] (pre-swapped) so each offset mul reads both SBUF inputs at
    the SAME base partition (BIR constraint) while the output is offset.
    4 DVE ops, no ACT."""
    t1 = pool.tile([128, 512], F32, tag="t1", name="t1", bufs=1)
    nc.vector.tensor_mul(t1[:], p[:], cosS[:])
    t2 = pool.tile([128, 512], F32, tag="t2", name="t2", bufs=1)
    nc.vector.tensor_mul(t2[0:64, :], p[64:128, :], sinS[64:128, :])
    nc.vector.tensor_mul(t2[64:128, :], p[0:64, :], sinS[0:64, :])
    nc.vector.tensor_add(dst[:], t1[:], t2[:])


def _emit_sb(nc, tensors, pools, sb, part):
    """Phase A for one seq block.  part: 'both' | 'v' | 'qk' (sb7 split).
    Weights are already streaming into w_r; xr tiles come from the xin pool.
    qk psum -> psb tiles (drained on DVE for sb7, ACT otherwise) + RoPE on
    DVE; v psum -> v_nat via ACT."""
    xT = tensors["xT"]
    cosf, sinf = tensors["cosf"], tensors["sinf"]
    w_r = tensors["w_r"]
    qT, kT, v_nat = tensors["qT"], tensors["kT"], tensors["v_nat"]
    xin, trig, rope, psQK, psV = (
        pools["xin"],
        pools["trig"],
        pools["rope"],
        pools.get("psQK"),
        pools.get("psV"),
    )
    sl = slice(sb * 512, (sb + 1) * 512)

    do_qk = part in ("both", "qk")
    do_v = part in ("both", "v")

    if do_qk:
        cosS = trig.tile([128, 512], F32, tag="cosS", name="cosS")
        sinS = trig.tile([128, 512], F32, tag="sinS", name="sinS")
        if sb > 0:  # sb0's tables load after its xr stream (startup path)
            nc.sync.dma_start(out=cosS[:], in_=cosf[:, sl])
            nc.sync.dma_start(out=sinS[:], in_=sinf[:, sl])
        ps = {
            t: psQK.tile([128, 512], F32, tag=f"ps_{t}", name=f"ps_{t}")
            for t in ("q0", "q1", "k0", "k1")
        }
    if do_v:
        vps = [
            psV.tile([128, 512], F32, tag=f"ps_v{j2}", name=f"ps_v{j2}")
            for j2 in range(2)
        ]

    for d in range(NDT):
        xr = xin.tile([128, 512], BF16, tag="xr", name="xr")
        nc.sync.dma_start(out=xr[:], in_=xT[d * 128 : (d + 1) * 128, sl])
        st = dict(start=(d == 0), stop=(d == NDT - 1))
        if do_qk:
            for h in range(HPC):
                co = d * CPC + h * HD
                nc.tensor.matmul(
                    ps[f"q{h}"][:], w_r["q"][:, co : co + HD], xr[:], **st
                )
                nc.tensor.matmul(
                    ps[f"k{h}"][:], w_r["k"][:, co : co + HD], xr[:], **st
                )
        if do_v:
            for j in range(4):
                # two j-chunks share a psum bank: only the bank's first
                # matmul clears has_written (start-once), the second chunk's
                # d==0 matmul lands on clear bits and overwrites per-element
                nc.tensor.matmul(
                    vps[j // 2][:, (j % 2) * 256 : (j % 2) * 256 + 256],
                    xr[:, j * 128 : (j + 1) * 128],
                    w_r["v"][:, d * CPC : (d + 1) * CPC],
                    start=(d == 0 and j % 2 == 0),
                    stop=(d == NDT - 1),
                    skip_group_check=True,
                )

    if do_qk and sb == 0:
        nc.sync.dma_start(out=cosS[:], in_=cosf[:, sl])
        nc.sync.dma_start(out=sinS[:], in_=sinf[:, sl])
    if do_v:
        for j2 in range(2):
            nc.scalar.copy(v_nat[sb][:, j2 * 512 : (j2 + 1) * 512], vps[j2][:])
    if do_qk:
        psb = {}
        for h in range(HPC):
            for t in (f"q{h}", f"k{h}"):
                pt = rope.tile([128, 512], F32, tag=f"psb_{t}", name=f"psb_{t}", bufs=1)
                if part == "qk":  # sb7: ACT is needed for phase-B exps
                    nc.vector.tensor_copy(pt[:], ps[t][:])
                else:
                    nc.scalar.copy(pt[:], ps[t][:])
                psb[t] = pt
        return psb, cosS, sinS
    return None, None, None


def _emit_rope_sb(nc, tensors, pools, sb, psb, cosS, sinS, heads):
    qT, kT = tensors["qT"], tensors["kT"]
    rope = pools["rope"]
    for h in heads:
        _rope_pair(nc, qT[h][sb], psb[f"q{h}"], cosS, sinS, rope)
        _rope_pair(nc, kT[h][sb], psb[f"k{h}"], cosS, sinS, rope)


class _PhaseB:
    """Per-(head, q-block) causal attention with 2-group exp lookahead,
    width-restricted diagonal, pairwise denominator tree, gpsimd all-reduce
    normalization, and deferred flush for a2a overlap."""

    def __init__(self, nc, tensors, pools, a2a_ins, a2a_outs, groups, a2a_r):
        self.nc = nc
        self.t = tensors
        self.p = pools
        self.a2a_ins = a2a_ins
        self.a2a_outs = a2a_outs
        self.groups = groups
        self.a2a_r = a2a_r
        self.pending = None  # awaiting flush1 (all-reduce kickoff)
        self.pending2 = []  # deque awaiting flush2 (recip/at/dma), max 2

    def flush1(self):
        """Kick the gpsimd all-reduce for the previous qb.  Only gpsimd-queue
        ops here -- nothing that could head-of-line block DVE."""
        nc = self.nc
        if self.pending is None:
            return
        fh, fqb, fpo, fprS = self.pending
        self.pending = None
        nrm = self.p["nrm"]
        denf = nrm.tile([128, 512], F32, tag="denf", name="denf", bufs=2)
        nc.gpsimd.partition_all_reduce(
            denf[:], fprS[:], channels=128, reduce_op=bass_isa.ReduceOp.add
        )
        self.pending2.append((self.gk, fh, fqb, fpo, denf))

    def flush2(self):
        """DVE tail of the flush, emitted >=6 groups after flush1 so the
        reciprocal never waits on the all-reduce in the DVE FIFO."""
        nc = self.nc
        if not self.pending2:
            return
        _, fh, fqb, fpo, denf = self.pending2.pop(0)
        nrm, attnP, a2asb = self.p["nrm"], self.p["attn"], self.p["a2asb"]
        rcsf = nrm.tile([128, 512], F32, tag="rcsf", name="rcsf")
        nc.vector.reciprocal_approx_fast(out=rcsf[:], in_=denf[:])
        at = attnP.tile([128, 512], BF16, tag="at", name="at")
        nc.vector.tensor_mul(at[:], fpo[:], rcsf[:])
        # gpsimd dma queue: never behind collective-gated transfers (the
        # sync queue carries the ar staging, which waits on the AllToAll)
        nc.gpsimd.dma_start(
            out=self.a2a_ins[fh][fqb * 128 : (fqb + 1) * 128, :], in_=at[:]
        )
        if fqb == 0:  # qb0 is processed last per head
            nc.gpsimd.collective_compute(
                "AllToAll",
                mybir.AluOpType.bypass,
                replica_groups=self.groups,
                ins=[self.a2a_ins[fh].opt()],
                outs=[self.a2a_outs[fh].opt()],
            )
            ar = a2asb.tile(
                [128, NCORES * 512], BF16, tag=f"a2r{fh}", name=f"a2r{fh}"
            )
            for c in range(NCORES):
                nc.sync.dma_start(
                    out=ar[:, c * 512 : (c + 1) * 512],
                    in_=self.a2a_outs[fh][c * 128 : (c + 1) * 128, :],
                )
            self.a2a_r.append(ar)

    @staticmethod
    def _kt_off(qb, g, half):
        kt = 2 * g + half
        if kt >= 4 * qb:  # diagonal 512-block
            return (kt - 4 * qb) * 128
        return 0

    def _emit_sc(self, item):
        nc = self.nc
        h, qb, g, G = item
        kT, qT = self.t["kT"], self.t["qT"]
        ss2 = self.p["psS"].tile([128, 1024], F32, tag="ss2", name="ss2")
        self.ss2s[(h, qb, g)] = ss2
        for half in range(2):
            kt = 2 * g + half
            off = self._kt_off(qb, g, half)
            diag = kt >= 4 * qb
            nc.tensor.matmul(
                ss2[:, half * 512 + off : (half + 1) * 512],
                kT[h][kt // 4][:, (kt % 4) * 128 : (kt % 4 + 1) * 128],
                qT[h][qb][:, off:512],
                start=True,
                stop=not diag,
                skip_group_check=diag,
            )
            if diag:
                # accumulate -1e9 into the strictly-upper triangle of the
                # first 128 q-cols (PE-side causal mask; exp -> exact 0)
                nc.tensor.matmul(
                    ss2[:, half * 512 + off : half * 512 + off + 128],
                    self.t["ident"][:],
                    self.t["trineg"][:],
                    start=False,
                    stop=True,
                    skip_group_check=True,
                )

    def _emit_exp(self, item):
        nc = self.nc
        h, qb, g, G = item
        probs = self.p["probs"]
        ss2 = self.ss2s.pop((h, qb, g))
        diag = self._kt_off(qb, g, 1) > 0
        if not diag:
            pr2 = probs.tile([128, 1024], BF16, tag="pr2", name="pr2", bufs=4)
            nc.scalar.activation(
                pr2[:], ss2[:], mybir.ActivationFunctionType.Exp, scale=SCALE
            )
        else:
            dt_tag = "pr2d0" if self._kt_off(qb, g, 0) == 0 else "pr2d1"
            pr2 = probs.tile([128, 1024], BF16, tag=dt_tag, name=dt_tag, bufs=2)
            for half in range(2):
                off = self._kt_off(qb, g, half)
                nc.scalar.activation(
                    pr2[:, half * 512 + off : (half + 1) * 512],
                    ss2[:, half * 512 + off : (half + 1) * 512],
                    mybir.ActivationFunctionType.Exp,
                    scale=SCALE,
                )
        self.pr2s[(h, qb, g)] = pr2

    def _emit_pv(self, item):
        nc = self.nc
        h, qb, g, G = item
        v_nat = self.t["v_nat"]
        if g == 0:
            self.po[(h, qb)] = self.p["psO"].tile(
                [128, 512], F32, tag="po", name="po", bufs=2
            )
        po = self.po[(h, qb)]
        pr2 = self.pr2s[(h, qb, g)]
        for half in range(2):
            kt = 2 * g + half
            off = self._kt_off(qb, g, half)
            nc.tensor.matmul(
                po[:, off:512],
                v_nat[kt // 4][
                    :, ((kt % 4) * 2 + h) * 128 : ((kt % 4) * 2 + h + 1) * 128
                ],
                pr2[:, half * 512 + off : (half + 1) * 512],
                start=(g == 0 and half == 0),
                stop=(g == G - 1 and half == 1),
                skip_group_check=True,
            )

    def _tree_push(self, item):
        nc = self.nc
        h, qb, g, G = item
        treeP = self.p["tree"]
        if g % 2 == 1:
            tree = self.tree.setdefault((h, qb), [])
            s = treeP.tile([128, 1024], BF16, tag="tl0", name="tl0", bufs=2)
            nc.vector.tensor_add(
                s[:],
                self.pr2s.pop((h, qb, g - 1))[:],
                self.pr2s.pop((h, qb, g))[:],
            )
            tree.append((1, s))
            while len(tree) >= 2 and tree[-1][0] == tree[-2][0]:
                l1, t1 = tree.pop()
                _, t2 = tree.pop()
                tn = treeP.tile(
                    [128, 1024], BF16, tag=f"tl{l1}", name=f"tl{l1}", bufs=2
                )
                nc.vector.tensor_add(tn[:], t1[:], t2[:])
                tree.append((l1 + 1, tn))
        if g == G - 1:  # finalize qb: collapse + fold, hand off to flush1
            tree = self.tree.pop((h, qb))
            while len(tree) > 1:
                l1, t1 = tree.pop()
                _, t2 = tree.pop()
                tn = treeP.tile([128, 1024], BF16, tag="tlc", name="tlc", bufs=2)
                nc.vector.tensor_add(tn[:], t1[:], t2[:])
                tree.append((l1 + 1, tn))
            root = tree[0][1]
            prS = self.p["fold"].tile([128, 512], BF16, tag="prS", name="prS")
            nc.vector.tensor_add(prS[:], root[:, 0:512], root[:, 512:1024])
            self.pending = (h, qb, self.po.pop((h, qb)), prS)

    def run(self, entries):
        """entries: (h, qb) tuples and callables (side work for other engine
        queues).  One continuous group pipeline -- the score/exp lookahead
        crosses qb boundaries so the PE/ACT pipeline never resets."""
        self.ss2s, self.pr2s, self.po, self.tree = {}, {}, {}, {}
        stream = []
        for e in entries:
            if callable(e):
                stream.append(e)
            else:
                h, qb = e
                G = 2 * qb + 2
                stream.extend((h, qb, g, G) for g in range(G))
        glist = [x for x in stream if not callable(x)]
        self._emit_sc(glist[0])
        self._emit_exp(glist[0])
        if len(glist) > 1:
            self._emit_sc(glist[1])
            self._emit_exp(glist[1])
        k = 0
        for item in stream:
            if callable(item):
                item()
                continue
            h, qb, g, G = item
            self.gk = k
            if g == 0:
                # po-bank WAR: at() of qb two-back must be emitted before
                # this qb's first PV reuses its psum slot
                while len(self.pending2) >= 2:
                    self.flush2()
            if g == 1:
                self.flush1()
            if self.pending2 and k - self.pending2[0][0] >= 6:
                self.flush2()
            if k + 2 < len(glist):
                self._emit_sc(glist[k + 2])
            self._emit_pv(item)
            self._tree_push(item)
            if k + 2 < len(glist):
                self._emit_exp(glist[k + 2])
            k += 1
        self.gk = k
        self.flush1()
        while self.pending2:
            self.flush2()


def _emit_wo_dma(nc, tensors, pools, h, dbp, c, queue):
    """One wo row-block DMA into the wosb pool (16 slots; h0's 16 tiles are
    prefetched on the gpsimd queue during phase B, h1's on ACT during C)."""
    wosb = pools["wosb"]
    wo_full = tensors["wo_full"]
    wr = wosb.tile([128, 1024], BF16, tag="wr", name="wr", bufs=13)
    ro = c * CPC + h * HD
    queue.dma_start(
        out=wr[:], in_=wo_full[ro : ro + 128, dbp * 1024 : (dbp + 1) * 1024]
    )
    return wr


def _phase_c(nc, tensors, pools, a2a_r, out_part, wrs):
    """Out-projection, ss-serial: per (h, dbp, j, ss) an 8-matmul psum group
    over source cores; h0 drains to bf16 SBUF acc, h1 adds + streams out."""
    psC, oacc, osb = (
        pools["psC"],
        pools["oacc"],
        pools["osb"],
    )
    # remaining wo rows stream in on the ACT dma queue while h0 computes
    for h in range(HPC):
        for dbp in range(2):
            for c in range(NCORES):
                if (h, dbp, c) not in wrs:
                    wrs[(h, dbp, c)] = _emit_wo_dma(
                        nc, tensors, pools, h, dbp, c, nc.scalar
                    )

    acc = {}
    for h in range(HPC):
        for dbp in range(2):
            for j in range(2):
                db = dbp * 2 + j
                for ss in range(4):
                    pc = psC.tile([128, 512], F32, tag="pc", name="pc", bufs=4)
                    for c in range(NCORES):
                        nc.tensor.matmul(
                            pc[:],
                            a2a_r[h][
                                :, c * 512 + ss * 128 : c * 512 + (ss + 1) * 128
                            ],
                            wrs[(h, dbp, c)][:, j * 512 : (j + 1) * 512],
                            start=(c == 0),
                            stop=(c == NCORES - 1),
                        )
                    if h == 0:
                        a = oacc.tile(
                            [128, 512], BF16, tag=f"acc{ss}{db}", name=f"acc{ss}{db}"
                        )
                        nc.scalar.copy(a[:], pc[:])
                        acc[(ss, db)] = a
                    else:
                        oc = osb.tile([128, 512], F32, tag="oc", name="oc")
                        nc.vector.tensor_add(oc[:], pc[:], acc[(ss, db)][:])
                        nc.sync.dma_start(
                            out=out_part[
                                ss * 128 : (ss + 1) * 128, db * 512 : (db + 1) * 512
                            ],
                            in_=oc[:],
                        )


def _build_program():
    nc = bacc.Bacc("TRN2", target_bir_lowering=False)

    tensors = {}
    tensors["xT"] = nc.dram_tensor("xT", [D, S], BF16, kind="ExternalInput")
    tensors["wq2"] = nc.dram_tensor("wq2", [D, CPC], BF16, kind="ExternalInput")
    tensors["wk2"] = nc.dram_tensor("wk2", [D, CPC], BF16, kind="ExternalInput")
    tensors["wv2"] = nc.dram_tensor("wv2", [D, CPC], BF16, kind="ExternalInput")
    tensors["wo_full"] = nc.dram_tensor("wo_full", [D, D], BF16, kind="ExternalInput")
    tensors["cosf"] = nc.dram_tensor("cosf", [HD, S], F32, kind="ExternalInput")
    tensors["sinf"] = nc.dram_tensor("sinf", [HD, S], F32, kind="ExternalInput")
    out_part = nc.dram_tensor("out_part", [S // NCORES, D], F32, kind="ExternalOutput")

    groups = [list(range(NCORES))]

    with TileContext(nc) as tc:
        with (
            tc.tile_pool(name="persist", bufs=1) as persist,
            tc.tile_pool(name="dram", bufs=1, space="DRAM") as dram,
            tc.tile_pool(name="xin", bufs=6) as xin,
            tc.tile_pool(name="trig", bufs=2) as trig,
            tc.tile_pool(name="rope", bufs=2) as rope,
            tc.tile_pool(name="probs", bufs=2) as probs,
            tc.tile_pool(name="tree", bufs=2) as treeP,
            tc.tile_pool(name="fold", bufs=2) as fold,
            tc.tile_pool(name="nrm", bufs=1) as nrm,
            tc.tile_pool(name="attn", bufs=2) as attnP,
            tc.tile_pool(name="a2asb", bufs=1) as a2asb,
            tc.tile_pool(name="wosb", bufs=16) as wosb,
            tc.tile_pool(name="oacc", bufs=1) as oacc,
            tc.tile_pool(name="osb", bufs=2) as osb,
        ):
            tensors["qT"] = [
                [
                    persist.tile([128, 512], BF16, tag=f"qT{h}_{sb}", name=f"qT{h}_{sb}")
                    for sb in range(NSB)
                ]
                for h in range(HPC)
            ]
            tensors["kT"] = [
                [
                    persist.tile([128, 512], BF16, tag=f"kT{h}_{sb}", name=f"kT{h}_{sb}")
                    for sb in range(NSB)
                ]
                for h in range(HPC)
            ]
            tensors["v_nat"] = [
                persist.tile([128, 1024], BF16, tag=f"v_nat{sb}", name=f"v_nat{sb}")
                for sb in range(NSB)
            ]
            # weights in SBUF, loaded in chunks on the ACT dma queue
            tensors["w_r"] = {
                wname: persist.tile(
                    [128, NDT * CPC], BF16, tag=f"w{wname}r", name=f"w{wname}r"
                )
                for wname in ("q", "k", "v")
            }

            # statically-zeroed garbage regions of the diagonal prob tiles
            # (allocated once per buf slot; exps never write these columns)
            for buf in range(2):
                pd0 = probs.tile([128, 1024], BF16, tag="pr2d0", name="pr2d0", bufs=2)
                nc.gpsimd.memset(pd0[:, 512:640], 0.0)
                pd1 = probs.tile([128, 1024], BF16, tag="pr2d1", name="pr2d1", bufs=2)
                nc.gpsimd.memset(pd1[:, 0:256], 0.0)
                nc.gpsimd.memset(pd1[:, 512:896], 0.0)

            # PE-side causal triangle for diagonal 128-blocks:
            # ident^T @ trineg adds -1e9 where q-col < key-row
            trineg = persist.tile([128, 128], BF16, tag="trineg", name="trineg")
            nc.any.memset(trineg[:], 0.0)
            nc.gpsimd.affine_select(
                out=trineg[:],
                in_=trineg[:],
                compare_op=mybir.AluOpType.is_ge,
                fill=-1.0e9,
                base=0,
                pattern=[[1, 128]],
                channel_multiplier=-1,
            )
            tensors["trineg"] = trineg
            ident = persist.tile([128, 128], BF16, tag="ident", name="ident")
            nc.any.memset(ident[:], 1.0)
            nc.gpsimd.affine_select(
                out=ident[:],
                in_=ident[:],
                compare_op=mybir.AluOpType.is_ge,
                fill=0.0,
                base=0,
                pattern=[[1, 128]],
                channel_multiplier=-1,
            )
            nc.gpsimd.affine_select(
                out=ident[:],
                in_=ident[:],
                compare_op=mybir.AluOpType.is_ge,
                fill=0.0,
                base=0,
                pattern=[[-1, 128]],
                channel_multiplier=1,
            )
            tensors["ident"] = ident

            a2a_ins = [
                dram.tile([NCORES * 128, 512], BF16, tag=f"a2i{h}", name=f"a2i{h}")
                for h in range(HPC)
            ]
            a2a_outs = [
                dram.tile([NCORES * 128, 512], BF16, tag=f"a2o{h}", name=f"a2o{h}")
                for h in range(HPC)
            ]

            pools = {
                "xin": xin,
                "trig": trig,
                "rope": rope,
                "probs": probs,
                "tree": treeP,
                "fold": fold,
                "nrm": nrm,
                "attn": attnP,
                "a2asb": a2asb,
            }

            def load_w_chunk(g, ndt, only=None):
                # one queue per weight tensor so the streams run in parallel:
                # wq on ACT, wv on gpsimd, wk interleaved on sync (via wk_cb)
                for wname, eng in (
                    ("q", nc.scalar),
                    ("k", nc.gpsimd),
                    ("v", nc.gpsimd),
                ):
                    if only is not None and wname != only:
                        continue
                    wv_ = tensors[f"w{wname}2"].rearrange("(dt p) c -> p dt c", p=128)
                    eng.dma_start(
                        out=tensors["w_r"][wname][
                            :, g * CPC : (g + ndt) * CPC
                        ].rearrange("p (dt c) -> p dt c", dt=ndt),
                        in_=wv_[:, g : g + ndt, :],
                    )



            pools["wosb"] = wosb
            pools["oacc"] = oacc
            pools["osb"] = osb
            a2a_r = []
            with tc.tile_pool(name="psQK", bufs=1, space="PSUM") as psQK:
                pools["psQK"] = psQK
                with tc.tile_pool(name="psV", bufs=1, space="PSUM") as psV:
                    pools["psV"] = psV
                    for g, ndt in ((0, 1), (1, 3), (4, 4), (8, 4), (12, 4)):
                        load_w_chunk(g, ndt, only="q")
                        load_w_chunk(g, ndt, only="k")
                        load_w_chunk(g, ndt, only="v")
                    # sb0..sb6 fused qk+v
                    for sb in range(NSB - 1):
                        psb, cosS, sinS = _emit_sb(nc, tensors, pools, sb, "both")
                        _emit_rope_sb(
                            nc, tensors, pools, sb, psb, cosS, sinS, range(HPC)
                        )
                    # sb7 v-pass, then psV closes
                    _emit_sb(nc, tensors, pools, 7, "v")
                # sb7 qk-pass (psQK still open; drains go to DVE)
                psb7, cosS7, sinS7 = _emit_sb(nc, tensors, pools, 7, "qk")
            # psQK released -> psO(2) + psS(6) take the 8 banks; psO stays
            # open through phase C so psC never zone-depends on the po/at tail
            wrs = {}
            with tc.tile_pool(name="psO", bufs=2, space="PSUM") as psO:
              pools["psO"] = psO
              with tc.tile_pool(name="psS", bufs=3, space="PSUM") as psS:
                pools["psS"] = psS
                pb = _PhaseB(nc, tensors, pools, a2a_ins, a2a_outs, groups, a2a_r)

                def wo_pre(k2):
                    # 2 wo DMAs on the (mostly idle) gpsimd dma queue; only
                    # h0's 16 tiles prefetch here, so the 16-slot pool never
                    # forces a WAR wait on this queue mid-phase-B
                    for k in (2 * k2, 2 * k2 + 1):
                        if k < 13:
                            dbp, c = divmod(k, NCORES)
                            wrs[(0, dbp, c)] = _emit_wo_dma(
                                nc, tensors, pools, 0, dbp, c, nc.gpsimd
                            )

                # B-early: qb5 ready after sb6; overlaps sb7's drains
                entries = [
                    (0, 5),
                    lambda: _emit_rope_sb(
                        nc, tensors, pools, 7, psb7, cosS7, sinS7, [0]
                    ),
                    lambda: wo_pre(0),
                    (0, 6),
                    lambda: wo_pre(1),
                    (0, 7),
                    lambda: wo_pre(2),
                    (0, 1),
                    lambda: _emit_rope_sb(
                        nc, tensors, pools, 7, psb7, cosS7, sinS7, [1]
                    ),
                    lambda: wo_pre(3),
                ]
                for i, qb in enumerate((2, 3, 4, 0)):
                    entries.append((0, qb))
                    entries.append(lambda i=i: wo_pre(4 + i))
                for i, qb in enumerate((1, 2, 3, 4, 5, 6, 7, 0)):
                    entries.append((1, qb))
                    entries.append(lambda i=i: wo_pre(8 + i))
                pb.run(entries)
            with tc.tile_pool(name="psC", bufs=1, space="PSUM") as psC:
                pools["psC"] = psC
                _phase_c(nc, tensors, pools, a2a_r, out_part, wrs)

    nc.compile()
    return nc


_NC_CACHE = None


def _get_program():
    global _NC_CACHE
    if _NC_CACHE is None:
        _NC_CACHE = _build_program()
    return _NC_CACHE


def _rope_tables():
    # match reference's f32 arithmetic
    i = np.arange(0, HD, 2, dtype=np.float32) / np.float32(HD)
    freqs = (np.float32(1.0) / np.float32(ROPE_THETA) ** i).astype(np.float32)  # [64]
    ang = np.arange(S, dtype=np.float32)[:, None] * freqs[None, :]  # [S, 64]
    cos = np.cos(ang).astype(np.float32).T  # [64, S]
    sin = np.sin(ang).astype(np.float32).T
    cosf = np.concatenate([cos, cos], axis=0)  # [128, S]
    # pre-swapped for the partition-offset rope muls: rows [0:64] multiply
    # p[64:128] reading sinS[64:128] = -sin, rows [64:128] read [0:64] = +sin
    sinf = np.concatenate([sin, -sin], axis=0)
    return np.ascontiguousarray(cosf), np.ascontiguousarray(sinf)


def kernel(x, mask, wq, wk, wv, wo):
    # mask is the standard causal mask produced by setup_inputs; causality is
    # implemented directly in the device program, so the tensor itself is not
    # shipped to the cores.
    import ml_dtypes

    x = np.asarray(x, dtype=np.float32)
    wq = np.asarray(wq, dtype=np.float32)
    wk = np.asarray(wk, dtype=np.float32)
    wv = np.asarray(wv, dtype=np.float32)
    wo_b = np.ascontiguousarray(np.asarray(wo, dtype=np.float32).astype(ml_dtypes.bfloat16))

    xT = np.ascontiguousarray(
        x.reshape(S, D).T.astype(ml_dtypes.bfloat16)
    )  # [D, S] bf16

    # de-interleave permutation within each head (RoPE pairs -> halves)
    idx = np.concatenate([np.arange(0, HD, 2), np.arange(1, HD, 2)])
    perm = np.concatenate([h * HD + idx for h in range(H)])
    wq_p = wq[:, perm]
    wk_p = wk[:, perm]

    cosf, sinf = _rope_tables()

    nc = _get_program()
    in_maps = []
    for c in range(NCORES):
        csl = slice(c * CPC, (c + 1) * CPC)
        in_maps.append(
            {
                "xT": xT,
                "wq2": np.ascontiguousarray(wq_p[:, csl].astype(ml_dtypes.bfloat16)),
                "wk2": np.ascontiguousarray(wk_p[:, csl].astype(ml_dtypes.bfloat16)),
                "wv2": np.ascontiguousarray(wv[:, csl].astype(ml_dtypes.bfloat16)),
                "wo_full": wo_b,
                "cosf": cosf,
                "sinf": sinf,
            }
        )
    res = run_bass_kernel_spmd(nc, in_maps, core_ids=list(range(NCORES)))
    out = np.concatenate([res.results[c]["out_part"] for c in range(NCORES)], axis=0)
    return out.reshape(1, S, D).astype(np.float32)
